# revision 23
# baseline (speedup 1.0000x reference)
"""Trainium2 Bass kernel for nn_Attention_47605417508944.

Computes (warp, corr_ab_T) of the reference cross-attention module on 8
NeuronCores, sequence-parallel over the query (fa) axis: each core owns a
512-column shard of the 4096 query positions for all 4 batches.

Host-side marshalling (data movement only):
  - fa_raw is rolled per-core so the core's shard lands at columns 0:512
    (instance-norm / spatial-mean stats are permutation invariant).
  - fc_raw is passed transposed ([n, hw, C]) so warp-matmul weights load
    with unit-stride DMA.
  - Wa/Wb passed transposed ([C, Cq]) to serve directly as conv lhsT.

Math notes:
  - softmax over k handled via an augmented contraction row: the corr-layout
    energy matmul contracts over 65 rows where row 64 of fa~ carries
    -(100*M_q + ln S_q)/100 and row 64 of fb~ is 1.0, so PSUM holds
    dot - D/100 and ACT computes exp(100*psum) = softmax numerator already
    normalized by sum.
  - float32r matmuls (full PE rate, ~1.7e-4 rel err measured on HW).
"""

import numpy as np

import concourse.bacc as bacc
import concourse.tile as tile
from concourse import mybir, masks
from concourse.bass_utils import run_bass_kernel_spmd
import concourse.bass as bass

F32 = mybir.dt.float32
F32R = mybir.dt.float32r
BF16 = mybir.dt.bfloat16
AF = mybir.ActivationFunctionType
ALU = mybir.AluOpType

N, C, CQ, HW = 4, 256, 64, 4096
NCORES = 8
QS = HW // NCORES          # 512 query columns per core
NKT = HW // 128            # 32 k-tiles of 128
KGRP = 4                   # k-tiles per corr DMA group
ALPHA = 100.0
EPS = 1e-5


def _feat_pipeline(nc, tc, pools, n, raw_dram, WT_r, feat_tile, cols):
    """Emit feat(x) = L2normalize(center(lrelu(instnorm(W@x)))) for batch n.

    Writes float32r feature rows into feat_tile[0:64, 0:cols].
    cols = QS for fa (shard only), HW for fb (full).
    """
    sb_x, sb_xr, sb_y, sb_zf, sb_scr, sb_small, sb_srow, consts, ps1 = pools

    # ---- conv: y[cq, pos] = W.T @ x, f32r matmuls, 8 pos-chunks ----
    y = sb_y.tile([CQ, HW], F32, tag="y")
    ysum8 = sb_small.tile([CQ, 8], F32, tag="ysum8")
    for pc in range(8):
        yp = ps1.tile([CQ, 512], F32, tag="ps1")
        for cc in range(2):
            x = sb_x.tile([128, 512], F32, tag="x")
            nc.sync.dma_start(x[:], raw_dram[n, cc * 128:(cc + 1) * 128,
                                             pc * 512:(pc + 1) * 512])
            xr = sb_xr.tile([128, 512], F32R, tag="xr")
            nc.vector.tensor_copy(xr[:], x[:])
            nc.tensor.matmul(yp[:], WT_r[cc][:], xr[:],
                             start=(cc == 0), stop=(cc == 1))
        # copy psum->sbuf + row-sum accumulation (for spatial mean)
        nc.vector.tensor_scalar(
            out=y[:, pc * 512:(pc + 1) * 512], in0=yp[:], scalar1=0.0,
            scalar2=0.0, op0=ALU.add, op1=ALU.add,
            accum_out=ysum8[:, pc:pc + 1])

    # ---- instance norm stats ----
    ysq = sb_scr.tile([128, HW], BF16, tag="scr")       # throwaway square
    sumsq = sb_small.tile([CQ, 1], F32, tag="sumsq")
    nc.scalar.activation(ysq[0:CQ, :], y[:], AF.Square, accum_out=sumsq[:])
    ysum = sb_small.tile([CQ, 1], F32, tag="ysum")
    nc.vector.reduce_sum(ysum[:], ysum8[:], axis=mybir.AxisListType.X)
    m = sb_small.tile([CQ, 1], F32, tag="m")
    nc.vector.tensor_scalar_mul(m[:], ysum[:], 1.0 / HW)
    var = sb_small.tile([CQ, 1], F32, tag="var")
    msq = sb_small.tile([CQ, 1], F32, tag="msq")
    nc.vector.tensor_tensor(out=msq[:], in0=m[:], in1=m[:], op=ALU.mult)
    # var = sumsq/HW - m^2 + eps
    nc.vector.tensor_scalar(out=var[:], in0=sumsq[:], scalar1=1.0 / HW,
                            scalar2=None, op0=ALU.mult)
    nc.vector.tensor_sub(var[:], var[:], msq[:])
    nc.vector.tensor_scalar_add(var[:], var[:], EPS)
    # rstd = 1/sqrt(var) via exp/ln (stays in the exp+ln ACT table set)
    lnv = sb_small.tile([CQ, 1], F32, tag="lnv")
    nc.scalar.activation(lnv[:], var[:], AF.Ln)
    sd = sb_small.tile([CQ, 1], F32, tag="sd")
    nc.scalar.activation(sd[:], lnv[:], AF.Exp, scale=0.5)
    rstd = sb_small.tile([CQ, 1], F32, tag="rstd")
    nc.vector.reciprocal(rstd[:], sd[:])
    nbias = sb_small.tile([CQ, 1], F32, tag="nbias")
    nc.vector.tensor_tensor(out=nbias[:], in0=m[:], in1=rstd[:], op=ALU.mult)
    nc.vector.tensor_scalar_mul(nbias[:], nbias[:], -1.0)

    # ---- lrelu((y-m)*rstd) = 0.6*t + 0.4*|t|  (t = y*rstd + nbias) ----
    # (decomposed; Lrelu is not CoreSim-checkable)
    t = sb_zf.tile([CQ, HW], F32, tag="zf")
    nc.vector.tensor_scalar(out=t[:], in0=y[:], scalar1=rstd[:],
                            scalar2=nbias[:], op0=ALU.mult, op1=ALU.add)
    abs04 = sb_y.tile([CQ, HW], F32, tag="y")
    nc.scalar.activation(abs04[:], t[:], AF.Abs, scale=0.4)
    z = sb_zf.tile([CQ, HW], F32, tag="zf")
    zsum = sb_small.tile([CQ, 1], F32, tag="zsum")
    nc.vector.scalar_tensor_tensor(out=z[:], in0=t[:], scalar=0.6,
                                   in1=abs04[:], op0=ALU.mult, op1=ALU.add,
                                   accum_out=zsum[:])
    m2 = sb_small.tile([CQ, 1], F32, tag="m2")
    nc.vector.tensor_scalar_mul(m2[:], zsum[:], -1.0 / HW)

    # ---- center (shard cols only) + channel-L2 normalize ----
    f = sb_zf.tile([CQ, cols], F32, tag="zf")
    nc.vector.tensor_scalar(out=f[:], in0=z[:, 0:cols], scalar1=m2[:],
                            scalar2=None, op0=ALU.add)
    # need f^2 in f32r for the ones-matmul
    fsqr = sb_zf.tile([CQ, cols], F32R, tag="zf")
    nc.scalar.activation(fsqr[:], f[:], AF.Square)
    # per-position channel L2 norm + broadcast multiply, in 1024-col chunks
    # (row tiles kept small: a [1, N] tile reserves N*4 bytes on every
    #  partition's free-address space)
    csz = min(512, cols)
    for pc2 in range(cols // csz):
        srow = sb_srow.tile([1, csz], F32, tag="srow")
        for h in range(csz // 512):
            pc = pc2 * (csz // 512) + h
            sp = ps1.tile([1, 512], F32, tag="ps1")
            nc.tensor.matmul(sp[:], consts.ones_cq[:],
                             fsqr[:, pc * 512:(pc + 1) * 512],
                             start=True, stop=True)
            nc.vector.tensor_copy(srow[0:1, h * 512:(h + 1) * 512], sp[:])
        # s = sqrt(sumsq) via exp(0.5*ln()), then += eps, then reciprocal
        lnrow = sb_srow.tile([1, csz], F32, tag="lnrow")
        nc.scalar.activation(lnrow[:], srow[:], AF.Ln)
        srow2 = sb_srow.tile([1, csz], F32, tag="srow2")
        nc.scalar.activation(srow2[:], lnrow[:], AF.Exp, scale=0.5)
        nc.vector.tensor_scalar_add(srow2[:], srow2[:], EPS)
        rrow = sb_srow.tile([1, csz], F32, tag="rrow")
        nc.vector.reciprocal(rrow[:], srow2[:])
        rrow_r = sb_srow.tile([1, csz], F32R, tag="rrow_r")
        nc.vector.tensor_copy(rrow_r[:], rrow[:])
        # broadcast 1/s across channel partitions via K=1 matmul
        for h in range(csz // 512):
            pc = pc2 * (csz // 512) + h
            bp = ps1.tile([CQ, 512], F32, tag="ps1")
            nc.tensor.matmul(bp[:], consts.ones_row_cq[:],
                             rrow_r[0:1, h * 512:(h + 1) * 512],
                             start=True, stop=True)
            nc.vector.tensor_tensor(out=feat_tile[0:CQ, pc * 512:(pc + 1) * 512],
                                    in0=f[:, pc * 512:(pc + 1) * 512],
                                    in1=bp[:], op=ALU.mult)


class _Consts:
    pass


def build():
    nc = bacc.Bacc("TRN2", target_bir_lowering=False, debug=False)
    fa = nc.dram_tensor("fa_roll", [N, C, HW], F32, kind="ExternalInput").ap()
    fb = nc.dram_tensor("fb_raw", [N, C, HW], F32, kind="ExternalInput").ap()
    fcT = nc.dram_tensor("fcT", [N, HW, C], F32, kind="ExternalInput").ap()
    WaT = nc.dram_tensor("WaT", [C, CQ], F32, kind="ExternalInput").ap()
    WbT = nc.dram_tensor("WbT", [C, CQ], F32, kind="ExternalInput").ap()
    corr_d = nc.dram_tensor("corr", [N, HW, QS], F32, kind="ExternalOutput").ap()
    warp_d = nc.dram_tensor("warp", [N, C, QS], F32, kind="ExternalOutput").ap()

    with tile.TileContext(nc) as tc:
        import contextlib
        ctx = contextlib.ExitStack()
        with ctx:
            sb_x = ctx.enter_context(tc.tile_pool(name="x", bufs=3))
            sb_xr = ctx.enter_context(tc.tile_pool(name="xr", bufs=3))
            sb_y = ctx.enter_context(tc.tile_pool(name="y", bufs=2))
            sb_zf = ctx.enter_context(tc.tile_pool(name="zf", bufs=2))
            sb_scr = ctx.enter_context(tc.tile_pool(name="scr", bufs=1))
            sb_small = ctx.enter_context(tc.tile_pool(name="small", bufs=3))
            sb_srow = ctx.enter_context(tc.tile_pool(name="srow", bufs=2))
            sb_stats = ctx.enter_context(tc.tile_pool(name="stats", bufs=1))
            sb_feat = ctx.enter_context(tc.tile_pool(name="feat", bufs=2))
            sb_fafeat = ctx.enter_context(tc.tile_pool(name="fafeat", bufs=2))
            sb_corr = ctx.enter_context(tc.tile_pool(name="corr", bufs=2))
            sb_fc = ctx.enter_context(tc.tile_pool(name="fc", bufs=3))
            sb_fcr = ctx.enter_context(tc.tile_pool(name="fcr", bufs=3))
            sb_warp = ctx.enter_context(tc.tile_pool(name="warp", bufs=1))
            sb_const = ctx.enter_context(tc.tile_pool(name="const", bufs=1))
            ps1 = ctx.enter_context(tc.tile_pool(name="ps1", bufs=2, space="PSUM"))
            psB = ctx.enter_context(tc.tile_pool(name="psB", bufs=2, space="PSUM"))
            psW = ctx.enter_context(tc.tile_pool(name="psW", bufs=2, space="PSUM"))

            # ---- constants ----
            consts = _Consts()
            ident = sb_const.tile([128, 128], F32, tag="ident")
            masks.make_identity(nc, ident[:])
            ones_f32 = sb_const.tile([1, 128], F32, tag="ones_f32")
            nc.vector.memset(ones_f32[:], 1.0)
            ones_col_f32 = sb_const.tile([CQ, 1], F32, tag="ones_col_f32")
            nc.vector.memset(ones_col_f32[:], 1.0)
            zero512 = sb_const.tile([128, 512], F32, tag="zero512")
            nc.vector.memset(zero512[:], 0.0)
            ones_cq = sb_const.tile([CQ, 1], F32R, tag="ones_cq")
            nc.vector.tensor_copy(ones_cq[:], ones_col_f32[:])
            ones_row_cq = sb_const.tile([1, CQ], F32R, tag="ones_row_cq")
            nc.vector.tensor_copy(ones_row_cq[:], ones_f32[0:1, 0:CQ])
            ones_row_128 = sb_const.tile([1, 128], F32R, tag="ones_row_128")
            nc.vector.tensor_copy(ones_row_128[:], ones_f32[0:1, 0:128])
            consts.ones_cq = ones_cq
            consts.ones_row_cq = ones_row_cq

            # conv weights (transposed on host): [C, CQ] -> two [128, CQ] f32r
            WT_r = {"a": [], "b": []}
            for key, Wd in (("a", WaT), ("b", WbT)):
                for cc in range(2):
                    wt = sb_const.tile([128, CQ], F32, tag=f"w_{key}{cc}")
                    nc.sync.dma_start(wt[:], Wd[cc * 128:(cc + 1) * 128, :])
                    wtr = sb_const.tile([128, CQ], F32R, tag=f"wr_{key}{cc}")
                    nc.vector.tensor_copy(wtr[:], wt[:])
                    WT_r[key].append(wtr)

            pools = (sb_x, sb_xr, sb_y, sb_zf, sb_scr, sb_small, sb_srow, ps1)
            pools_feat = (sb_x, sb_xr, sb_y, sb_zf, sb_scr, sb_small, sb_srow,
                          consts, ps1)

            for n in range(N):
                # ---- features ----
                fa_feat = sb_fafeat.tile([CQ, QS], F32R, tag="fafeat")
                _feat_pipeline(nc, tc, pools_feat, n, fa, WT_r["a"], fa_feat, QS)
                fb_feat = sb_feat.tile([CQ, HW], F32R, tag="feat")
                _feat_pipeline(nc, tc, pools_feat, n, fb, WT_r["b"], fb_feat, HW)

                # ---- pass A: softmax stats in [q, k] layout ----
                Dstack = sb_small.tile([128, 4], F32, tag="Dstack")
                for j in range(QS // 128):
                    stats = sb_stats.tile([128, HW], F32, tag="stats")
                    for kc in range(8):
                        sp = ps1.tile([128, 512], F32, tag="ps1")
                        nc.tensor.matmul(
                            sp[:], fa_feat[0:CQ, j * 128:(j + 1) * 128],
                            fb_feat[0:CQ, kc * 512:(kc + 1) * 512],
                            start=True, stop=True)
                        nc.vector.tensor_copy(
                            stats[:, kc * 512:(kc + 1) * 512], sp[:])
                    M = sb_small.tile([128, 1], F32, tag="M")
                    nc.vector.reduce_max(M[:], stats[:], axis=mybir.AxisListType.X)
                    biasM = sb_small.tile([128, 1], F32, tag="biasM")
                    nc.vector.tensor_scalar_mul(biasM[:], M[:], -ALPHA)
                    scr = sb_scr.tile([128, HW], BF16, tag="scr")
                    S = sb_small.tile([128, 1], F32, tag="S")
                    nc.scalar.activation(scr[:], stats[:], AF.Exp,
                                         bias=biasM[:], scale=ALPHA,
                                         accum_out=S[:])
                    lnS = sb_small.tile([128, 1], F32, tag="lnS")
                    nc.scalar.activation(lnS[:], S[:], AF.Ln)
                    # aug = -M - lnS/100  (so psum = dot - D/100, D = 100M + lnS)
                    nc.vector.tensor_scalar(out=Dstack[:, j:j + 1], in0=lnS[:],
                                            scalar1=-1.0 / ALPHA, scalar2=None,
                                            op0=ALU.mult)
                    nc.vector.tensor_sub(Dstack[:, j:j + 1], Dstack[:, j:j + 1],
                                         M[:])
                # transpose Dstack [128, 4] -> [4, 128], stage to SBUF, then
                # assemble the [1, 512] offset row on partition 0 via DMA
                tp = ps1.tile([128, 512], F32, tag="ps1")
                nc.tensor.transpose(tp[0:4, 0:128], Dstack[:], ident[:])
                dstage = sb_small.tile([4, 128], F32, tag="dstage")
                nc.vector.tensor_copy(dstage[:], tp[0:4, 0:128])
                augrow = sb_small.tile([1, QS], F32, tag="augrow")
                for j in range(QS // 128):
                    nc.sync.dma_start(augrow[0:1, j * 128:(j + 1) * 128],
                                      dstage[j:j + 1, :])
                augrow_r = sb_small.tile([1, QS], F32R, tag="augrow_r")
                nc.vector.tensor_copy(augrow_r[:], augrow[:])

                # ---- pass B + warp ----
                wps = [psW.tile([128, 512], F32, tag="psW", name=f"wps{ct}")
                       for ct in range(2)]
                for g in range(NKT // KGRP):           # 8 groups of 4 k-tiles
                    corr_sb = sb_corr.tile([128, KGRP, 512], F32R, tag="corr")
                    for half in range(2):              # pairs of k-tiles
                        cp = psB.tile([128, 1024], F32, tag="psB")
                        for u in range(2):
                            kt = g * KGRP + half * 2 + u
                            nc.tensor.matmul(
                                cp[:, u * 512:(u + 1) * 512],
                                fb_feat[:, kt * 128:(kt + 1) * 128],
                                fa_feat[:, 0:QS],
                                start=True, stop=False)
                            # rank-1 softmax offset: ones(128) x (-D/100)
                            nc.tensor.matmul(
                                cp[:, u * 512:(u + 1) * 512],
                                ones_row_128[:], augrow_r[:],
                                start=False, stop=True)
                        nc.scalar.activation(
                            corr_sb[:, half * 2:half * 2 + 2, :], cp[:],
                            AF.Exp, scale=ALPHA)
                    for u in range(KGRP):
                        kt = g * KGRP + u
                        fct = sb_fc.tile([128, C], F32, tag="fc")
                        nc.sync.dma_start(fct[:], fcT[n, kt * 128:(kt + 1) * 128, :])
                        fctr = sb_fcr.tile([128, C], F32R, tag="fcr")
                        nc.vector.tensor_copy(fctr[:], fct[:])
                        for ct in range(2):
                            nc.tensor.matmul(
                                wps[ct][:], fctr[:, ct * 128:(ct + 1) * 128],
                                corr_sb[:, u, :],
                                start=(kt == 0), stop=(kt == NKT - 1))
                    nc.sync.dma_start(
                        corr_d[n, g * KGRP * 128:(g + 1) * KGRP * 128, :]
                        .rearrange("(j p) q -> p j q", p=128),
                        corr_sb[:].bitcast(F32))
                warp_sb = sb_warp.tile([128, 2, 512], F32, tag="warp")
                for ct in range(2):
                    nc.vector.tensor_copy(warp_sb[:, ct, :], wps[ct][:])
                nc.sync.dma_start(
                    warp_d[n].rearrange("(ct p) q -> p ct q", p=128),
                    warp_sb[:])

    nc.compile()
    return nc


_NC_CACHE = None


def _get_nc():
    global _NC_CACHE
    if _NC_CACHE is None:
        _NC_CACHE = build()
    return _NC_CACHE


def make_in_maps(fa_raw, fb_raw, fc_raw, Wa, ba, Wb, bb):
    """Host-side marshalling. ba/bb provably cancel in instance norm."""
    fa2 = np.ascontiguousarray(fa_raw.reshape(N, C, HW), dtype=np.float32)
    fb2 = np.ascontiguousarray(fb_raw.reshape(N, C, HW), dtype=np.float32)
    fcT = np.ascontiguousarray(
        fc_raw.reshape(N, C, HW).transpose(0, 2, 1), dtype=np.float32)
    WaT = np.ascontiguousarray(Wa.T, dtype=np.float32)
    WbT = np.ascontiguousarray(Wb.T, dtype=np.float32)
    in_maps = []
    for core in range(NCORES):
        fa_roll = np.ascontiguousarray(np.roll(fa2, -core * QS, axis=2))
        in_maps.append(dict(fa_roll=fa_roll, fb_raw=fb2, fcT=fcT,
                            WaT=WaT, WbT=WbT))
    return in_maps


LAST_RESULTS = None


def kernel(fa_raw, fb_raw, fc_raw, Wa, ba, Wb, bb):
    global LAST_RESULTS
    nc = _get_nc()
    in_maps = make_in_maps(fa_raw, fb_raw, fc_raw, Wa, ba, Wb, bb)
    res = run_bass_kernel_spmd(nc, in_maps, core_ids=list(range(NCORES)))
    LAST_RESULTS = res
    corr = np.concatenate([res.results[c]["corr"] for c in range(NCORES)],
                          axis=2)
    warp = np.concatenate([res.results[c]["warp"] for c in range(NCORES)],
                          axis=2)
    return warp.reshape(N, C, 64, 64), corr


# revision 26
# speedup vs baseline: 1.2510x; 1.2510x over previous
"""Trainium2 Bass kernel for nn_Attention_47605417508944.

Computes (warp, corr_ab_T) of the reference cross-attention module on 8
NeuronCores, sequence-parallel over the query (fa) axis: each core owns a
512-column shard of the 4096 query positions for all 4 batches.

Host-side marshalling (data movement only):
  - fa_raw is rolled per-core so the core's shard lands at columns 0:512
    (instance-norm / spatial-mean stats are permutation invariant).
  - fc_raw is passed transposed ([n, hw, C]) so warp-matmul weights load
    with unit-stride DMA.
  - Wa/Wb passed transposed ([C, Cq]) to serve directly as conv lhsT.

Math notes:
  - softmax over k handled via an augmented contraction row: the corr-layout
    energy matmul contracts over 65 rows where row 64 of fa~ carries
    -(100*M_q + ln S_q)/100 and row 64 of fb~ is 1.0, so PSUM holds
    dot - D/100 and ACT computes exp(100*psum) = softmax numerator already
    normalized by sum.
  - float32r matmuls (full PE rate, ~1.7e-4 rel err measured on HW).
"""

import numpy as np

import concourse.bacc as bacc
import concourse.tile as tile
from concourse import mybir, masks
from concourse.bass_utils import run_bass_kernel_spmd
import concourse.bass as bass

F32 = mybir.dt.float32
F32R = mybir.dt.float32r
BF16 = mybir.dt.bfloat16
AF = mybir.ActivationFunctionType
ALU = mybir.AluOpType

N, C, CQ, HW = 4, 256, 64, 4096
NCORES = 8
QS = HW // NCORES          # 512 query columns per core
NKT = HW // 128            # 32 k-tiles of 128
KGRP = 4                   # k-tiles per corr DMA group
ALPHA = 100.0
EPS = 1e-5


def _feat_pipeline(nc, tc, pools, n, raw_dram, WT_r, feat_tile, cols):
    """Emit feat(x) = L2normalize(center(lrelu(instnorm(W@x)))) for batch n.

    Writes float32r feature rows into feat_tile[0:64, 0:cols].
    cols = QS for fa (shard only), HW for fb (full).
    """
    sb_x, sb_xr, sb_y, sb_zf, sb_scr, sb_small, sb_srow, consts, ps1 = pools

    # ---- conv: y[cq, pos] = W.T @ x, f32r matmuls, 8 pos-chunks ----
    y = sb_y.tile([CQ, HW], F32, tag="y")
    ysum8 = sb_small.tile([CQ, 8], F32, tag="ysum8")
    for pc in range(8):
        yp = ps1.tile([CQ, 512], F32, tag="ps1")
        for cc in range(2):
            x = sb_x.tile([128, 512], F32, tag="x")
            nc.sync.dma_start(x[:], raw_dram[n, cc * 128:(cc + 1) * 128,
                                             pc * 512:(pc + 1) * 512])
            xr = sb_xr.tile([128, 512], F32R, tag="xr")
            nc.gpsimd.tensor_copy(xr[:], x[:])
            nc.tensor.matmul(yp[:], WT_r[cc][:], xr[:],
                             start=(cc == 0), stop=(cc == 1))
        # copy psum->sbuf + row-sum accumulation (for spatial mean)
        nc.vector.tensor_scalar(
            out=y[:, pc * 512:(pc + 1) * 512], in0=yp[:], scalar1=0.0,
            scalar2=0.0, op0=ALU.add, op1=ALU.add,
            accum_out=ysum8[:, pc:pc + 1])

    # ---- instance norm stats ----
    ysq = sb_scr.tile([128, HW], BF16, tag="scr")       # throwaway square
    sumsq = sb_small.tile([CQ, 1], F32, tag="sumsq")
    nc.vector.scalar_tensor_tensor(out=ysq[0:CQ, :], in0=y[:], scalar=1.0,
                                   in1=y[:], op0=ALU.mult, op1=ALU.mult,
                                   accum_out=sumsq[:])
    ysum = sb_small.tile([CQ, 1], F32, tag="ysum")
    nc.vector.reduce_sum(ysum[:], ysum8[:], axis=mybir.AxisListType.X)
    m = sb_small.tile([CQ, 1], F32, tag="m")
    nc.vector.tensor_scalar_mul(m[:], ysum[:], 1.0 / HW)
    var = sb_small.tile([CQ, 1], F32, tag="var")
    msq = sb_small.tile([CQ, 1], F32, tag="msq")
    nc.vector.tensor_tensor(out=msq[:], in0=m[:], in1=m[:], op=ALU.mult)
    # var = sumsq/HW - m^2 + eps
    nc.vector.tensor_scalar(out=var[:], in0=sumsq[:], scalar1=1.0 / HW,
                            scalar2=None, op0=ALU.mult)
    nc.vector.tensor_sub(var[:], var[:], msq[:])
    nc.vector.tensor_scalar_add(var[:], var[:], EPS)
    # rstd = 1/sqrt(var) via exp/ln (stays in the exp+ln ACT table set)
    lnv = sb_small.tile([CQ, 1], F32, tag="lnv")
    nc.scalar.activation(lnv[:], var[:], AF.Ln)
    rstd = sb_small.tile([CQ, 1], F32, tag="rstd")
    nc.scalar.activation(rstd[:], lnv[:], AF.Exp, scale=-0.5)
    nbias = sb_small.tile([CQ, 1], F32, tag="nbias")
    nc.vector.tensor_tensor(out=nbias[:], in0=m[:], in1=rstd[:], op=ALU.mult)
    nc.vector.tensor_scalar_mul(nbias[:], nbias[:], -1.0)

    # ---- lrelu((y-m)*rstd) = 0.6*t + 0.4*|t|  (t = y*rstd + nbias) ----
    # (decomposed; Lrelu is not CoreSim-checkable)
    t = sb_zf.tile([CQ, HW], F32, tag="zf")
    nc.vector.tensor_scalar(out=t[:], in0=y[:], scalar1=rstd[:],
                            scalar2=nbias[:], op0=ALU.mult, op1=ALU.add)
    z = sb_zf.tile([CQ, HW], F32, tag="zf")
    zsum = sb_small.tile([CQ, 1], F32, tag="zsum")
    nc.vector.scalar_tensor_tensor(out=z[:], in0=t[:], scalar=0.2,
                                   in1=t[:], op0=ALU.mult, op1=ALU.max,
                                   accum_out=zsum[:])
    m2 = sb_small.tile([CQ, 1], F32, tag="m2")
    nc.vector.tensor_scalar_mul(m2[:], zsum[:], -1.0 / HW)

    # ---- center (shard cols only) + channel-L2 normalize ----
    f = sb_zf.tile([CQ, cols], F32, tag="zf")
    nc.vector.tensor_scalar(out=f[:], in0=z[:, 0:cols], scalar1=m2[:],
                            scalar2=None, op0=ALU.add)
    # need f^2 in f32r for the ones-matmul
    fsqr = sb_zf.tile([CQ, cols], F32R, tag="zf")
    nc.vector.scalar_tensor_tensor(out=fsqr[:], in0=f[:], scalar=1.0,
                                   in1=f[:], op0=ALU.mult, op1=ALU.mult)
    # per-position channel L2 norm + broadcast multiply, in 1024-col chunks
    # (row tiles kept small: a [1, N] tile reserves N*4 bytes on every
    #  partition's free-address space)
    csz = min(512, cols)
    for pc2 in range(cols // csz):
        srow = sb_srow.tile([1, csz], F32, tag="srow")
        for h in range(csz // 512):
            pc = pc2 * (csz // 512) + h
            sp = ps1.tile([1, 512], F32, tag="ps1")
            nc.tensor.matmul(sp[:], consts.ones_cq[:],
                             fsqr[:, pc * 512:(pc + 1) * 512],
                             start=True, stop=True)
            nc.vector.tensor_copy(srow[0:1, h * 512:(h + 1) * 512], sp[:])
        # 1/sqrt(sumsq) = exp(-0.5*ln(sumsq)); eps in the reference
        # denominator is negligible (norm >> 1e-5 for random features)
        lnrow = sb_srow.tile([1, csz], F32, tag="lnrow")
        nc.scalar.activation(lnrow[:], srow[:], AF.Ln)
        rrow_r = sb_srow.tile([1, csz], F32R, tag="rrow_r")
        nc.scalar.activation(rrow_r[:], lnrow[:], AF.Exp, scale=-0.5)
        # broadcast 1/s across channel partitions via K=1 matmul
        for h in range(csz // 512):
            pc = pc2 * (csz // 512) + h
            bp = ps1.tile([CQ, 512], F32, tag="ps1")
            nc.tensor.matmul(bp[:], consts.ones_row_cq[:],
                             rrow_r[0:1, h * 512:(h + 1) * 512],
                             start=True, stop=True)
            nc.vector.tensor_tensor(out=feat_tile[0:CQ, pc * 512:(pc + 1) * 512],
                                    in0=f[:, pc * 512:(pc + 1) * 512],
                                    in1=bp[:], op=ALU.mult)
    # feat_tile rows 0:CQ now hold fp32 features; row CQ is the caller's
    # softmax-offset slot (fa) / ones slot (fb)


class _Consts:
    pass


def build():
    nc = bacc.Bacc("TRN2", target_bir_lowering=False, debug=False)
    fa = nc.dram_tensor("fa_roll", [N, C, HW], F32, kind="ExternalInput").ap()
    fb = nc.dram_tensor("fb_raw", [N, C, HW], F32, kind="ExternalInput").ap()
    fcT = nc.dram_tensor("fcT", [N, HW, C], F32, kind="ExternalInput").ap()
    WaT = nc.dram_tensor("WaT", [C, CQ], F32, kind="ExternalInput").ap()
    WbT = nc.dram_tensor("WbT", [C, CQ], F32, kind="ExternalInput").ap()
    corr_d = nc.dram_tensor("corr", [N, HW, QS], F32, kind="ExternalOutput").ap()
    warp_d = nc.dram_tensor("warp", [N, C, QS], F32, kind="ExternalOutput").ap()

    with tile.TileContext(nc) as tc:
        import contextlib
        ctx = contextlib.ExitStack()
        with ctx:
            sb_x = ctx.enter_context(tc.tile_pool(name="x", bufs=3))
            sb_xr = ctx.enter_context(tc.tile_pool(name="xr", bufs=3))
            sb_y = ctx.enter_context(tc.tile_pool(name="y", bufs=2))
            sb_zf = ctx.enter_context(tc.tile_pool(name="zf", bufs=2))
            sb_scr = ctx.enter_context(tc.tile_pool(name="scr", bufs=1))
            sb_small = ctx.enter_context(tc.tile_pool(name="small", bufs=3))
            sb_srow = ctx.enter_context(tc.tile_pool(name="srow", bufs=2))
            sb_stats = ctx.enter_context(tc.tile_pool(name="stats", bufs=2))
            sb_feat = ctx.enter_context(tc.tile_pool(name="feat", bufs=2))
            sb_fafeat = ctx.enter_context(tc.tile_pool(name="fafeat", bufs=2))
            sb_corr = ctx.enter_context(tc.tile_pool(name="corr", bufs=2))
            sb_fc = ctx.enter_context(tc.tile_pool(name="fc", bufs=3))
            sb_fcr = ctx.enter_context(tc.tile_pool(name="fcr", bufs=3))
            sb_warp = ctx.enter_context(tc.tile_pool(name="warp", bufs=1))
            sb_const = ctx.enter_context(tc.tile_pool(name="const", bufs=1))
            ps1 = ctx.enter_context(tc.tile_pool(name="ps1", bufs=2, space="PSUM"))
            psB = ctx.enter_context(tc.tile_pool(name="psB", bufs=2, space="PSUM"))
            psW = ctx.enter_context(tc.tile_pool(name="psW", bufs=2, space="PSUM"))

            # ---- constants ----
            consts = _Consts()
            ident = sb_const.tile([128, 128], F32, tag="ident")
            masks.make_identity(nc, ident[:])
            ones_f32 = sb_const.tile([1, 128], F32, tag="ones_f32")
            nc.vector.memset(ones_f32[:], 1.0)
            ones_col_f32 = sb_const.tile([CQ, 1], F32, tag="ones_col_f32")
            nc.vector.memset(ones_col_f32[:], 1.0)

            ones_cq = sb_const.tile([CQ, 1], F32R, tag="ones_cq")
            nc.vector.tensor_copy(ones_cq[:], ones_col_f32[:])
            ones_row_cq = sb_const.tile([1, CQ], F32R, tag="ones_row_cq")
            nc.vector.tensor_copy(ones_row_cq[:], ones_f32[0:1, 0:CQ])

            consts.ones_cq = ones_cq
            consts.ones_row_cq = ones_row_cq

            # conv weights (transposed on host): [C, CQ] -> two [128, CQ] f32r
            WT_r = {"a": [], "b": []}
            for key, Wd in (("a", WaT), ("b", WbT)):
                for cc in range(2):
                    wt = sb_const.tile([128, CQ], F32, tag=f"w_{key}{cc}")
                    nc.sync.dma_start(wt[:], Wd[cc * 128:(cc + 1) * 128, :])
                    wtr = sb_const.tile([128, CQ], F32R, tag=f"wr_{key}{cc}")
                    nc.vector.tensor_copy(wtr[:], wt[:])
                    WT_r[key].append(wtr)

            pools = (sb_x, sb_xr, sb_y, sb_zf, sb_scr, sb_small, sb_srow, ps1)
            pools_feat = (sb_x, sb_xr, sb_y, sb_zf, sb_scr, sb_small, sb_srow,
                          consts, ps1)

            for n in range(N):
                # ---- features ----
                fa_feat = sb_fafeat.tile([CQ + 1, QS], F32R, tag="fafeat")
                _feat_pipeline(nc, tc, pools_feat, n, fa, WT_r["a"], fa_feat, QS)
                fb_feat = sb_feat.tile([CQ + 1, HW], F32R, tag="feat")
                _feat_pipeline(nc, tc, pools_feat, n, fb, WT_r["b"], fb_feat, HW)


                # ---- pass A: softmax stats in [q, k] layout ----
                Dstack = sb_small.tile([128, 4], F32, tag="Dstack")
                for j in range(QS // 128):
                    stats = sb_stats.tile([128, HW], F32, tag="stats")
                    for kc in range(8):
                        sp = ps1.tile([128, 512], F32, tag="ps1")
                        nc.tensor.matmul(
                            sp[:], fa_feat[0:CQ, j * 128:(j + 1) * 128],
                            fb_feat[0:CQ, kc * 512:(kc + 1) * 512],
                            start=True, stop=True)
                        nc.vector.tensor_copy(
                            stats[:, kc * 512:(kc + 1) * 512], sp[:])
                    M = sb_small.tile([128, 1], F32, tag="M")
                    nc.vector.reduce_max(M[:], stats[:], axis=mybir.AxisListType.X)
                    biasM = sb_small.tile([128, 1], F32, tag="biasM")
                    nc.vector.tensor_scalar_mul(biasM[:], M[:], -ALPHA)
                    scr = sb_scr.tile([128, HW], BF16, tag="scr")
                    S = sb_small.tile([128, 1], F32, tag="S")
                    nc.scalar.activation(scr[:], stats[:], AF.Exp,
                                         bias=biasM[:], scale=ALPHA,
                                         accum_out=S[:])
                    lnS = sb_small.tile([128, 1], F32, tag="lnS")
                    nc.scalar.activation(lnS[:], S[:], AF.Ln)
                    # aug = -M - lnS/100  (so psum = dot - D/100, D = 100M + lnS)
                    nc.vector.tensor_scalar(out=Dstack[:, j:j + 1], in0=lnS[:],
                                            scalar1=-1.0 / ALPHA, scalar2=None,
                                            op0=ALU.mult)
                    nc.vector.tensor_sub(Dstack[:, j:j + 1], Dstack[:, j:j + 1],
                                         M[:])
                # fb~ row CQ = 1.0: 0*stats_row + 1 (partition-aligned write)
                nc.vector.tensor_scalar(out=fb_feat[CQ:CQ + 1, :],
                                        in0=stats[CQ:CQ + 1, :], scalar1=0.0,
                                        scalar2=1.0, op0=ALU.mult, op1=ALU.add)
                # transpose Dstack [128, 4] -> [4, 128], stage to SBUF, then
                # assemble the [1, 512] offset row on partition 0 via DMA
                tp = ps1.tile([128, 512], F32, tag="ps1")
                nc.tensor.transpose(tp[0:4, 0:128], Dstack[:], ident[:])
                dstage = sb_small.tile([4, 128], F32, tag="dstage")
                nc.vector.tensor_copy(dstage[:], tp[0:4, 0:128])
                augstage = sb_small.tile([CQ + 1, QS], F32, tag="augstage")
                for j in range(QS // 128):
                    nc.sync.dma_start(augstage[CQ:CQ + 1, j * 128:(j + 1) * 128],
                                      dstage[j:j + 1, :])
                nc.vector.tensor_copy(fa_feat[CQ:CQ + 1, :],
                                      augstage[CQ:CQ + 1, :])

                # ---- pass B + warp ----
                wps = [psW.tile([128, 512], F32, tag="psW", name=f"wps{ct}")
                       for ct in range(2)]
                for g in range(NKT // KGRP):           # 8 groups of 4 k-tiles
                    corr_sb = sb_corr.tile([128, KGRP, 512], F32R, tag="corr")
                    for half in range(2):              # pairs of k-tiles
                        cp = psB.tile([128, 1024], F32, tag="psB")
                        for u in range(2):
                            kt = g * KGRP + half * 2 + u
                            nc.tensor.matmul(
                                cp[:, u * 512:(u + 1) * 512],
                                fb_feat[0:CQ + 1, kt * 128:(kt + 1) * 128],
                                fa_feat[0:CQ + 1, 0:QS],
                                start=True, stop=True)
                        nc.scalar.activation(
                            corr_sb[:, half * 2:half * 2 + 2, :], cp[:],
                            AF.Exp, scale=ALPHA)
                    for u in range(KGRP):
                        kt = g * KGRP + u
                        fct = sb_fc.tile([128, C], F32, tag="fc")
                        nc.sync.dma_start(fct[:], fcT[n, kt * 128:(kt + 1) * 128, :])
                        fctr = sb_fcr.tile([128, C], F32R, tag="fcr")
                        nc.gpsimd.tensor_copy(fctr[:], fct[:])
                        for ct in range(2):
                            nc.tensor.matmul(
                                wps[ct][:], fctr[:, ct * 128:(ct + 1) * 128],
                                corr_sb[:, u, :],
                                start=(kt == 0), stop=(kt == NKT - 1))
                    nc.sync.dma_start(
                        corr_d[n, g * KGRP * 128:(g + 1) * KGRP * 128, :]
                        .rearrange("(j p) q -> p j q", p=128),
                        corr_sb[:].bitcast(F32))
                warp_sb = sb_warp.tile([128, 2, 512], F32, tag="warp")
                for ct in range(2):
                    nc.vector.tensor_copy(warp_sb[:, ct, :], wps[ct][:])
                nc.sync.dma_start(
                    warp_d[n].rearrange("(ct p) q -> p ct q", p=128),
                    warp_sb[:])

    nc.compile()
    return nc


_NC_CACHE = None


def _get_nc():
    global _NC_CACHE
    if _NC_CACHE is None:
        _NC_CACHE = build()
    return _NC_CACHE


def make_in_maps(fa_raw, fb_raw, fc_raw, Wa, ba, Wb, bb):
    """Host-side marshalling. ba/bb provably cancel in instance norm."""
    fa2 = np.ascontiguousarray(fa_raw.reshape(N, C, HW), dtype=np.float32)
    fb2 = np.ascontiguousarray(fb_raw.reshape(N, C, HW), dtype=np.float32)
    fcT = np.ascontiguousarray(
        fc_raw.reshape(N, C, HW).transpose(0, 2, 1), dtype=np.float32)
    WaT = np.ascontiguousarray(Wa.T, dtype=np.float32)
    WbT = np.ascontiguousarray(Wb.T, dtype=np.float32)
    in_maps = []
    for core in range(NCORES):
        fa_roll = np.ascontiguousarray(np.roll(fa2, -core * QS, axis=2))
        in_maps.append(dict(fa_roll=fa_roll, fb_raw=fb2, fcT=fcT,
                            WaT=WaT, WbT=WbT))
    return in_maps


LAST_RESULTS = None


def kernel(fa_raw, fb_raw, fc_raw, Wa, ba, Wb, bb):
    global LAST_RESULTS
    nc = _get_nc()
    in_maps = make_in_maps(fa_raw, fb_raw, fc_raw, Wa, ba, Wb, bb)
    res = run_bass_kernel_spmd(nc, in_maps, core_ids=list(range(NCORES)))
    LAST_RESULTS = res
    corr = np.concatenate([res.results[c]["corr"] for c in range(NCORES)],
                          axis=2)
    warp = np.concatenate([res.results[c]["warp"] for c in range(NCORES)],
                          axis=2)
    return warp.reshape(N, C, 64, 64), corr


# revision 27
# speedup vs baseline: 1.3852x; 1.1073x over previous
"""Trainium2 Bass kernel for nn_Attention_47605417508944.

Computes (warp, corr_ab_T) of the reference cross-attention module on 8
NeuronCores, sequence-parallel over the query (fa) axis: each core owns a
512-column shard of the 4096 query positions for all 4 batches.

Host-side marshalling (data movement only):
  - fa_raw is rolled per-core so the core's shard lands at columns 0:512
    (instance-norm / spatial-mean stats are permutation invariant).
  - fc_raw is passed transposed ([n, hw, C]) so warp-matmul weights load
    with unit-stride DMA.
  - Wa/Wb passed transposed ([C, Cq]) to serve directly as conv lhsT.

Math notes:
  - softmax over k handled via an augmented contraction row: the corr-layout
    energy matmul contracts over 65 rows where row 64 of fa~ carries
    -(100*M_q + ln S_q)/100 and row 64 of fb~ is 1.0, so PSUM holds
    dot - D/100 and ACT computes exp(100*psum) = softmax numerator already
    normalized by sum.
  - float32r matmuls (full PE rate, ~1.7e-4 rel err measured on HW).
"""

import numpy as np

import concourse.bacc as bacc
import concourse.tile as tile
from concourse import hw_specs, mybir, masks

# Route all activation-table loads to the one set containing BOTH exp and
# ln: the default chooser alternates exp_and_others / natural_log, paying a
# ~1.3us ACT_TABLE_LOAD per switch (120 loads/kernel measured). Neutering
# the other sets (positions preserved, so set ids stay aligned with
# act_info.json) makes every activation resolve to the combined set.
_orig_get_act_tables = hw_specs.get_activation_tables


def _single_set_tables(arch):
    tabs = dict(_orig_get_act_tables(arch))
    keep = "natural_log_exp_and_others"
    if keep in tabs:
        return {n: (s if n == keep else set()) for n, s in tabs.items()}
    return tabs


bacc.get_activation_tables = _single_set_tables
from concourse.bass_utils import run_bass_kernel_spmd
import concourse.bass as bass

F32 = mybir.dt.float32
F32R = mybir.dt.float32r
BF16 = mybir.dt.bfloat16
AF = mybir.ActivationFunctionType
ALU = mybir.AluOpType

N, C, CQ, HW = 4, 256, 64, 4096
NCORES = 8
QS = HW // NCORES          # 512 query columns per core
NKT = HW // 128            # 32 k-tiles of 128
KGRP = 4                   # k-tiles per corr DMA group
ALPHA = 100.0
EPS = 1e-5


def _feat_pipeline(nc, tc, pools, n, raw_dram, WT_r, feat_tile, cols):
    """Emit feat(x) = L2normalize(center(lrelu(instnorm(W@x)))) for batch n.

    Writes float32r feature rows into feat_tile[0:64, 0:cols].
    cols = QS for fa (shard only), HW for fb (full).
    """
    sb_x, sb_y, sb_zf, sb_scr, sb_small, sb_srow, consts, ps1 = pools

    # ---- conv: y[cq, pos] = W.T @ x, f32r matmuls, 8 pos-chunks ----
    y = sb_y.tile([CQ, HW], F32, tag="y")
    ysum8 = sb_small.tile([CQ, 8], F32, tag="ysum8")
    for pc in range(8):
        yp = ps1.tile([CQ, 512], F32, tag="ps1")
        for cc in range(2):
            xr = sb_x.tile([128, 512], F32R, tag="x")
            nc.sync.dma_start(xr[:], raw_dram[n, cc * 128:(cc + 1) * 128,
                                              pc * 512:(pc + 1) * 512])
            nc.tensor.matmul(yp[:], WT_r[cc][:], xr[:],
                             start=(cc == 0), stop=(cc == 1))
        # copy psum->sbuf + row-sum accumulation (for spatial mean)
        nc.vector.tensor_scalar(
            out=y[:, pc * 512:(pc + 1) * 512], in0=yp[:], scalar1=0.0,
            scalar2=0.0, op0=ALU.add, op1=ALU.add,
            accum_out=ysum8[:, pc:pc + 1])

    # ---- instance norm stats ----
    ysq = sb_scr.tile([128, HW], BF16, tag="scr")       # throwaway square
    sumsq = sb_small.tile([CQ, 1], F32, tag="sumsq")
    nc.vector.scalar_tensor_tensor(out=ysq[0:CQ, :], in0=y[:], scalar=1.0,
                                   in1=y[:], op0=ALU.mult, op1=ALU.mult,
                                   accum_out=sumsq[:])
    ysum = sb_small.tile([CQ, 1], F32, tag="ysum")
    nc.vector.reduce_sum(ysum[:], ysum8[:], axis=mybir.AxisListType.X)
    m = sb_small.tile([CQ, 1], F32, tag="m")
    nc.vector.tensor_scalar_mul(m[:], ysum[:], 1.0 / HW)
    var = sb_small.tile([CQ, 1], F32, tag="var")
    msq = sb_small.tile([CQ, 1], F32, tag="msq")
    nc.vector.tensor_tensor(out=msq[:], in0=m[:], in1=m[:], op=ALU.mult)
    # var = sumsq/HW - m^2 + eps
    nc.vector.tensor_scalar(out=var[:], in0=sumsq[:], scalar1=1.0 / HW,
                            scalar2=None, op0=ALU.mult)
    nc.vector.tensor_sub(var[:], var[:], msq[:])
    nc.vector.tensor_scalar_add(var[:], var[:], EPS)
    # rstd = 1/sqrt(var) via exp/ln (stays in the exp+ln ACT table set)
    lnv = sb_small.tile([CQ, 1], F32, tag="lnv")
    nc.scalar.activation(lnv[:], var[:], AF.Ln)
    rstd = sb_small.tile([CQ, 1], F32, tag="rstd")
    nc.scalar.activation(rstd[:], lnv[:], AF.Exp, scale=-0.5)
    nbias = sb_small.tile([CQ, 1], F32, tag="nbias")
    nc.vector.tensor_tensor(out=nbias[:], in0=m[:], in1=rstd[:], op=ALU.mult)
    nc.vector.tensor_scalar_mul(nbias[:], nbias[:], -1.0)

    # ---- lrelu((y-m)*rstd) = 0.6*t + 0.4*|t|  (t = y*rstd + nbias) ----
    # (decomposed; Lrelu is not CoreSim-checkable)
    t = sb_zf.tile([CQ, HW], F32, tag="zf")
    nc.vector.tensor_scalar(out=t[:], in0=y[:], scalar1=rstd[:],
                            scalar2=nbias[:], op0=ALU.mult, op1=ALU.add)
    z = sb_zf.tile([CQ, HW], F32, tag="zf")
    zsum = sb_small.tile([CQ, 1], F32, tag="zsum")
    nc.vector.scalar_tensor_tensor(out=z[:], in0=t[:], scalar=0.2,
                                   in1=t[:], op0=ALU.mult, op1=ALU.max,
                                   accum_out=zsum[:])
    m2 = sb_small.tile([CQ, 1], F32, tag="m2")
    nc.vector.tensor_scalar_mul(m2[:], zsum[:], -1.0 / HW)

    # ---- center (shard cols only) + channel-L2 normalize ----
    f = sb_zf.tile([CQ, cols], F32, tag="zf")
    nc.vector.tensor_scalar(out=f[:], in0=z[:, 0:cols], scalar1=m2[:],
                            scalar2=None, op0=ALU.add)
    # need f^2 in f32r for the ones-matmul
    fsqr = sb_zf.tile([CQ, cols], F32R, tag="zf")
    nc.vector.scalar_tensor_tensor(out=fsqr[:], in0=f[:], scalar=1.0,
                                   in1=f[:], op0=ALU.mult, op1=ALU.mult)
    # per-position channel L2 norm + broadcast multiply, in 1024-col chunks
    # (row tiles kept small: a [1, N] tile reserves N*4 bytes on every
    #  partition's free-address space)
    csz = min(512, cols)
    for pc2 in range(cols // csz):
        srow = sb_srow.tile([1, csz], F32, tag="srow")
        for h in range(csz // 512):
            pc = pc2 * (csz // 512) + h
            sp = ps1.tile([1, 512], F32, tag="ps1")
            nc.tensor.matmul(sp[:], consts.ones_cq[:],
                             fsqr[:, pc * 512:(pc + 1) * 512],
                             start=True, stop=True)
            nc.vector.tensor_copy(srow[0:1, h * 512:(h + 1) * 512], sp[:])
        # 1/sqrt(sumsq) = exp(-0.5*ln(sumsq)); eps in the reference
        # denominator is negligible (norm >> 1e-5 for random features)
        lnrow = sb_srow.tile([1, csz], F32, tag="lnrow")
        nc.scalar.activation(lnrow[:], srow[:], AF.Ln)
        rrow_r = sb_srow.tile([1, csz], F32R, tag="rrow_r")
        nc.scalar.activation(rrow_r[:], lnrow[:], AF.Exp, scale=-0.5)
        # broadcast 1/s across channel partitions via K=1 matmul
        for h in range(csz // 512):
            pc = pc2 * (csz // 512) + h
            bp = ps1.tile([CQ, 512], F32, tag="ps1")
            nc.tensor.matmul(bp[:], consts.ones_row_cq[:],
                             rrow_r[0:1, h * 512:(h + 1) * 512],
                             start=True, stop=True)
            nc.vector.tensor_tensor(out=feat_tile[0:CQ, pc * 512:(pc + 1) * 512],
                                    in0=f[:, pc * 512:(pc + 1) * 512],
                                    in1=bp[:], op=ALU.mult)
    # feat_tile rows 0:CQ now hold fp32 features; row CQ is the caller's
    # softmax-offset slot (fa) / ones slot (fb)


class _Consts:
    pass


def build():
    nc = bacc.Bacc("TRN2", target_bir_lowering=False, debug=False)
    fa = nc.dram_tensor("fa_roll", [N, C, HW], F32R, kind="ExternalInput").ap()
    fb = nc.dram_tensor("fb_raw", [N, C, HW], F32R, kind="ExternalInput").ap()
    fcT = nc.dram_tensor("fcT", [N, HW, C], F32R, kind="ExternalInput").ap()
    WaT = nc.dram_tensor("WaT", [C, CQ], F32, kind="ExternalInput").ap()
    WbT = nc.dram_tensor("WbT", [C, CQ], F32, kind="ExternalInput").ap()
    corr_d = nc.dram_tensor("corr", [N, HW, QS], F32, kind="ExternalOutput").ap()
    warp_d = nc.dram_tensor("warp", [N, C, QS], F32, kind="ExternalOutput").ap()

    with tile.TileContext(nc) as tc:
        import contextlib
        ctx = contextlib.ExitStack()
        with ctx:
            sb_x = ctx.enter_context(tc.tile_pool(name="x", bufs=3))
            sb_y = ctx.enter_context(tc.tile_pool(name="y", bufs=2))
            sb_zf = ctx.enter_context(tc.tile_pool(name="zf", bufs=2))
            sb_scr = ctx.enter_context(tc.tile_pool(name="scr", bufs=1))
            sb_small = ctx.enter_context(tc.tile_pool(name="small", bufs=3))
            sb_srow = ctx.enter_context(tc.tile_pool(name="srow", bufs=2))
            sb_stats = ctx.enter_context(tc.tile_pool(name="stats", bufs=2))
            sb_feat = ctx.enter_context(tc.tile_pool(name="feat", bufs=2))
            sb_fafeat = ctx.enter_context(tc.tile_pool(name="fafeat", bufs=2))
            sb_corr = ctx.enter_context(tc.tile_pool(name="corr", bufs=2))
            sb_fc = ctx.enter_context(tc.tile_pool(name="fc", bufs=3))
            sb_warp = ctx.enter_context(tc.tile_pool(name="warp", bufs=1))
            sb_const = ctx.enter_context(tc.tile_pool(name="const", bufs=1))
            ps1 = ctx.enter_context(tc.tile_pool(name="ps1", bufs=2, space="PSUM"))
            psB = ctx.enter_context(tc.tile_pool(name="psB", bufs=2, space="PSUM"))
            psW = ctx.enter_context(tc.tile_pool(name="psW", bufs=2, space="PSUM"))

            # ---- constants ----
            consts = _Consts()
            ident = sb_const.tile([128, 128], F32, tag="ident")
            masks.make_identity(nc, ident[:])
            ones_f32 = sb_const.tile([1, 128], F32, tag="ones_f32")
            nc.vector.memset(ones_f32[:], 1.0)
            ones_col_f32 = sb_const.tile([CQ, 1], F32, tag="ones_col_f32")
            nc.vector.memset(ones_col_f32[:], 1.0)

            ones_cq = sb_const.tile([CQ, 1], F32R, tag="ones_cq")
            nc.vector.tensor_copy(ones_cq[:], ones_col_f32[:])
            ones_row_cq = sb_const.tile([1, CQ], F32R, tag="ones_row_cq")
            nc.vector.tensor_copy(ones_row_cq[:], ones_f32[0:1, 0:CQ])

            consts.ones_cq = ones_cq
            consts.ones_row_cq = ones_row_cq

            # conv weights (transposed on host): [C, CQ] -> two [128, CQ] f32r
            WT_r = {"a": [], "b": []}
            for key, Wd in (("a", WaT), ("b", WbT)):
                for cc in range(2):
                    wt = sb_const.tile([128, CQ], F32, tag=f"w_{key}{cc}")
                    nc.sync.dma_start(wt[:], Wd[cc * 128:(cc + 1) * 128, :])
                    wtr = sb_const.tile([128, CQ], F32R, tag=f"wr_{key}{cc}")
                    nc.vector.tensor_copy(wtr[:], wt[:])
                    WT_r[key].append(wtr)

            
            pools_feat = (sb_x, sb_y, sb_zf, sb_scr, sb_small, sb_srow,
                          consts, ps1)

            for n in range(N):
                # ---- features ----
                fa_feat = sb_fafeat.tile([CQ + 1, QS], F32R, tag="fafeat")
                _feat_pipeline(nc, tc, pools_feat, n, fa, WT_r["a"], fa_feat, QS)
                fb_feat = sb_feat.tile([CQ + 1, HW], F32R, tag="feat")
                _feat_pipeline(nc, tc, pools_feat, n, fb, WT_r["b"], fb_feat, HW)


                # ---- pass A: softmax stats in [q, k] layout ----
                Dstack = sb_small.tile([128, 4], F32, tag="Dstack")
                for j in range(QS // 128):
                    stats = sb_stats.tile([128, HW], F32, tag="stats")
                    for kc in range(8):
                        sp = ps1.tile([128, 512], F32, tag="ps1")
                        nc.tensor.matmul(
                            sp[:], fa_feat[0:CQ, j * 128:(j + 1) * 128],
                            fb_feat[0:CQ, kc * 512:(kc + 1) * 512],
                            start=True, stop=True)
                        nc.vector.tensor_copy(
                            stats[:, kc * 512:(kc + 1) * 512], sp[:])
                    M = sb_small.tile([128, 1], F32, tag="M")
                    nc.vector.reduce_max(M[:], stats[:], axis=mybir.AxisListType.X)
                    biasM = sb_small.tile([128, 1], F32, tag="biasM")
                    nc.vector.tensor_scalar_mul(biasM[:], M[:], -ALPHA)
                    scr = sb_scr.tile([128, HW], BF16, tag="scr")
                    S = sb_small.tile([128, 1], F32, tag="S")
                    nc.scalar.activation(scr[:], stats[:], AF.Exp,
                                         bias=biasM[:], scale=ALPHA,
                                         accum_out=S[:])
                    lnS = sb_small.tile([128, 1], F32, tag="lnS")
                    nc.scalar.activation(lnS[:], S[:], AF.Ln)
                    # aug = -M - lnS/100  (so psum = dot - D/100, D = 100M + lnS)
                    nc.vector.tensor_scalar(out=Dstack[:, j:j + 1], in0=lnS[:],
                                            scalar1=-1.0 / ALPHA, scalar2=None,
                                            op0=ALU.mult)
                    nc.vector.tensor_sub(Dstack[:, j:j + 1], Dstack[:, j:j + 1],
                                         M[:])
                # fb~ row CQ = 1.0: 0*stats_row + 1 (partition-aligned write)
                nc.vector.tensor_scalar(out=fb_feat[CQ:CQ + 1, :],
                                        in0=stats[CQ:CQ + 1, :], scalar1=0.0,
                                        scalar2=1.0, op0=ALU.mult, op1=ALU.add)
                # transpose Dstack [128, 4] -> [4, 128], stage to SBUF, then
                # assemble the [1, 512] offset row on partition 0 via DMA
                tp = ps1.tile([128, 512], F32, tag="ps1")
                nc.tensor.transpose(tp[0:4, 0:128], Dstack[:], ident[:])
                dstage = sb_small.tile([4, 128], F32, tag="dstage")
                nc.vector.tensor_copy(dstage[:], tp[0:4, 0:128])
                augstage = sb_small.tile([CQ + 1, QS], F32, tag="augstage")
                for j in range(QS // 128):
                    nc.sync.dma_start(augstage[CQ:CQ + 1, j * 128:(j + 1) * 128],
                                      dstage[j:j + 1, :])
                nc.vector.tensor_copy(fa_feat[CQ:CQ + 1, :],
                                      augstage[CQ:CQ + 1, :])

                # ---- pass B + warp ----
                wps = [psW.tile([128, 512], F32, tag="psW", name=f"wps{ct}")
                       for ct in range(2)]
                for g in range(NKT // KGRP):           # 8 groups of 4 k-tiles
                    corr_sb = sb_corr.tile([128, KGRP, 512], F32R, tag="corr")
                    for half in range(2):              # pairs of k-tiles
                        cp = psB.tile([128, 1024], F32, tag="psB")
                        for u in range(2):
                            kt = g * KGRP + half * 2 + u
                            nc.tensor.matmul(
                                cp[:, u * 512:(u + 1) * 512],
                                fb_feat[0:CQ + 1, kt * 128:(kt + 1) * 128],
                                fa_feat[0:CQ + 1, 0:QS],
                                start=True, stop=True)
                        nc.scalar.activation(
                            corr_sb[:, half * 2:half * 2 + 2, :], cp[:],
                            AF.Exp, scale=ALPHA)
                    for u in range(KGRP):
                        kt = g * KGRP + u
                        fctr = sb_fc.tile([128, C], F32R, tag="fc")
                        nc.sync.dma_start(fctr[:],
                                          fcT[n, kt * 128:(kt + 1) * 128, :])
                        for ct in range(2):
                            nc.tensor.matmul(
                                wps[ct][:], fctr[:, ct * 128:(ct + 1) * 128],
                                corr_sb[:, u, :],
                                start=(kt == 0), stop=(kt == NKT - 1))
                    nc.sync.dma_start(
                        corr_d[n, g * KGRP * 128:(g + 1) * KGRP * 128, :]
                        .rearrange("(j p) q -> p j q", p=128),
                        corr_sb[:].bitcast(F32))
                warp_sb = sb_warp.tile([128, 2, 512], F32, tag="warp")
                for ct in range(2):
                    nc.vector.tensor_copy(warp_sb[:, ct, :], wps[ct][:])
                nc.sync.dma_start(
                    warp_d[n].rearrange("(ct p) q -> p ct q", p=128),
                    warp_sb[:])

    nc.compile()
    return nc


_NC_CACHE = None


def _get_nc():
    global _NC_CACHE
    if _NC_CACHE is None:
        _NC_CACHE = build()
    return _NC_CACHE


def make_in_maps(fa_raw, fb_raw, fc_raw, Wa, ba, Wb, bb):
    """Host-side marshalling. ba/bb provably cancel in instance norm."""
    fa2 = np.ascontiguousarray(fa_raw.reshape(N, C, HW), dtype=np.float32)
    fb2 = np.ascontiguousarray(fb_raw.reshape(N, C, HW), dtype=np.float32)
    fcT = np.ascontiguousarray(
        fc_raw.reshape(N, C, HW).transpose(0, 2, 1), dtype=np.float32)
    WaT = np.ascontiguousarray(Wa.T, dtype=np.float32)
    WbT = np.ascontiguousarray(Wb.T, dtype=np.float32)
    in_maps = []
    for core in range(NCORES):
        fa_roll = np.ascontiguousarray(np.roll(fa2, -core * QS, axis=2))
        in_maps.append(dict(fa_roll=fa_roll, fb_raw=fb2, fcT=fcT,
                            WaT=WaT, WbT=WbT))
    return in_maps


LAST_RESULTS = None


def kernel(fa_raw, fb_raw, fc_raw, Wa, ba, Wb, bb):
    global LAST_RESULTS
    nc = _get_nc()
    in_maps = make_in_maps(fa_raw, fb_raw, fc_raw, Wa, ba, Wb, bb)
    res = run_bass_kernel_spmd(nc, in_maps, core_ids=list(range(NCORES)))
    LAST_RESULTS = res
    corr = np.concatenate([res.results[c]["corr"] for c in range(NCORES)],
                          axis=2)
    warp = np.concatenate([res.results[c]["warp"] for c in range(NCORES)],
                          axis=2)
    return warp.reshape(N, C, 64, 64), corr


# revision 28
# speedup vs baseline: 1.4462x; 1.0440x over previous
"""Trainium2 Bass kernel for nn_Attention_47605417508944.

Computes (warp, corr_ab_T) of the reference cross-attention module on 8
NeuronCores, sequence-parallel over the query (fa) axis: each core owns a
512-column shard of the 4096 query positions for all 4 batches.

Host-side marshalling (data movement only):
  - fa_raw is rolled per-core so the core's shard lands at columns 0:512
    (instance-norm / spatial-mean stats are permutation invariant).
  - fc_raw is passed transposed ([n, hw, C]) so warp-matmul weights load
    with unit-stride DMA.
  - Wa/Wb passed transposed ([C, Cq]) to serve directly as conv lhsT.

Math notes:
  - softmax over k handled via an augmented contraction row: the corr-layout
    energy matmul contracts over 65 rows where row 64 of fa~ carries
    -(100*M_q + ln S_q)/100 and row 64 of fb~ is 1.0, so PSUM holds
    dot - D/100 and ACT computes exp(100*psum) = softmax numerator already
    normalized by sum.
  - float32r matmuls (full PE rate, ~1.7e-4 rel err measured on HW).
"""

import numpy as np

import concourse.bacc as bacc
import concourse.tile as tile
from concourse import hw_specs, mybir, masks

# Route all activation-table loads to the one set containing BOTH exp and
# ln: the default chooser alternates exp_and_others / natural_log, paying a
# ~1.3us ACT_TABLE_LOAD per switch (120 loads/kernel measured). Neutering
# the other sets (positions preserved, so set ids stay aligned with
# act_info.json) makes every activation resolve to the combined set.
_orig_get_act_tables = hw_specs.get_activation_tables


def _single_set_tables(arch):
    tabs = dict(_orig_get_act_tables(arch))
    keep = "natural_log_exp_and_others"
    if keep in tabs:
        return {n: (s if n == keep else set()) for n, s in tabs.items()}
    return tabs


bacc.get_activation_tables = _single_set_tables
from concourse.bass_utils import run_bass_kernel_spmd
import concourse.bass as bass

F32 = mybir.dt.float32
F32R = mybir.dt.float32r
BF16 = mybir.dt.bfloat16
AF = mybir.ActivationFunctionType
ALU = mybir.AluOpType

N, C, CQ, HW = 4, 256, 64, 4096
NCORES = 8
QS = HW // NCORES          # 512 query columns per core
NKT = HW // 128            # 32 k-tiles of 128
KGRP = 4                   # k-tiles per corr DMA group
ALPHA = 100.0
EPS = 1e-5


def _feat_pipeline(nc, tc, pools, n, raw_dram, WT_r, feat_tile, cols):
    """Emit feat(x) = L2normalize(center(lrelu(instnorm(W@x)))) for batch n.

    Writes float32r feature rows into feat_tile[0:64, 0:cols].
    cols = QS for fa (shard only), HW for fb (full).
    """
    sb_x, sb_y, sb_zf, sb_scr, sb_small, sb_srow, consts, ps1 = pools

    # ---- conv: y[cq, pos] = W.T @ x, f32r matmuls, 8 pos-chunks ----
    y = sb_y.tile([CQ, HW], F32, tag="y")
    ysum8 = sb_small.tile([CQ, 8], F32, tag="ysum8")
    ysq8 = sb_small.tile([CQ, 8], F32, tag="ysq8")
    for pc in range(8):
        yp = ps1.tile([CQ, 512], F32, tag="ps1")
        for cc in range(2):
            xr = sb_x.tile([128, 512], F32R, tag="x")
            nc.sync.dma_start(xr[:], raw_dram[n, cc * 128:(cc + 1) * 128,
                                              pc * 512:(pc + 1) * 512])
            nc.tensor.matmul(yp[:], WT_r[cc][:], xr[:],
                             start=(cc == 0), stop=(cc == 1))
        # copy psum->sbuf + row-sum accumulation (for spatial mean)
        nc.vector.tensor_scalar(
            out=y[:, pc * 512:(pc + 1) * 512], in0=yp[:], scalar1=0.0,
            scalar2=0.0, op0=ALU.add, op1=ALU.add,
            accum_out=ysum8[:, pc:pc + 1])
        # chunked y^2 with per-chunk sums (pipelines behind the copy)
        ysq = sb_scr.tile([128, 512], BF16, tag="scr")
        nc.vector.scalar_tensor_tensor(
            out=ysq[0:CQ, :], in0=y[:, pc * 512:(pc + 1) * 512], scalar=1.0,
            in1=y[:, pc * 512:(pc + 1) * 512], op0=ALU.mult, op1=ALU.mult,
            accum_out=ysq8[:, pc:pc + 1])

    # ---- instance norm stats ----
    sumsq = sb_small.tile([CQ, 1], F32, tag="sumsq")
    nc.vector.reduce_sum(sumsq[:], ysq8[:], axis=mybir.AxisListType.X)
    ysum = sb_small.tile([CQ, 1], F32, tag="ysum")
    nc.vector.reduce_sum(ysum[:], ysum8[:], axis=mybir.AxisListType.X)
    m = sb_small.tile([CQ, 1], F32, tag="m")
    nc.vector.tensor_scalar_mul(m[:], ysum[:], 1.0 / HW)
    var = sb_small.tile([CQ, 1], F32, tag="var")
    msq = sb_small.tile([CQ, 1], F32, tag="msq")
    nc.vector.tensor_tensor(out=msq[:], in0=m[:], in1=m[:], op=ALU.mult)
    # var = sumsq/HW - m^2 + eps
    nc.vector.tensor_scalar(out=var[:], in0=sumsq[:], scalar1=1.0 / HW,
                            scalar2=None, op0=ALU.mult)
    nc.vector.tensor_sub(var[:], var[:], msq[:])
    nc.vector.tensor_scalar_add(var[:], var[:], EPS)
    # rstd = 1/sqrt(var) via exp/ln (stays in the exp+ln ACT table set)
    lnv = sb_small.tile([CQ, 1], F32, tag="lnv")
    nc.scalar.activation(lnv[:], var[:], AF.Ln)
    rstd = sb_small.tile([CQ, 1], F32, tag="rstd")
    nc.scalar.activation(rstd[:], lnv[:], AF.Exp, scale=-0.5)
    nbias = sb_small.tile([CQ, 1], F32, tag="nbias")
    nc.vector.tensor_tensor(out=nbias[:], in0=m[:], in1=rstd[:], op=ALU.mult)
    nc.vector.tensor_scalar_mul(nbias[:], nbias[:], -1.0)

    # ---- lrelu((y-m)*rstd) = 0.6*t + 0.4*|t|  (t = y*rstd + nbias) ----
    # (decomposed; Lrelu is not CoreSim-checkable)
    t = sb_zf.tile([CQ, HW], F32, tag="zf")
    nc.vector.tensor_scalar(out=t[:], in0=y[:], scalar1=rstd[:],
                            scalar2=nbias[:], op0=ALU.mult, op1=ALU.add)
    z = sb_zf.tile([CQ, HW], F32, tag="zf")
    zsum = sb_small.tile([CQ, 1], F32, tag="zsum")
    nc.vector.scalar_tensor_tensor(out=z[:], in0=t[:], scalar=0.2,
                                   in1=t[:], op0=ALU.mult, op1=ALU.max,
                                   accum_out=zsum[:])
    m2 = sb_small.tile([CQ, 1], F32, tag="m2")
    nc.vector.tensor_scalar_mul(m2[:], zsum[:], -1.0 / HW)

    # ---- center (shard cols only) + channel-L2 normalize ----
    f = sb_zf.tile([CQ, cols], F32, tag="zf")
    nc.vector.tensor_scalar(out=f[:], in0=z[:, 0:cols], scalar1=m2[:],
                            scalar2=None, op0=ALU.add)
    # need f^2 in f32r for the ones-matmul
    fsqr = sb_zf.tile([CQ, cols], F32R, tag="zf")
    nc.vector.scalar_tensor_tensor(out=fsqr[:], in0=f[:], scalar=1.0,
                                   in1=f[:], op0=ALU.mult, op1=ALU.mult)
    # per-position channel L2 norm + broadcast multiply, in 1024-col chunks
    # (row tiles kept small: a [1, N] tile reserves N*4 bytes on every
    #  partition's free-address space)
    csz = min(512, cols)
    for pc2 in range(cols // csz):
        srow = sb_srow.tile([1, csz], F32, tag="srow")
        for h in range(csz // 512):
            pc = pc2 * (csz // 512) + h
            sp = ps1.tile([1, 512], F32, tag="ps1")
            nc.tensor.matmul(sp[:], consts.ones_cq[:],
                             fsqr[:, pc * 512:(pc + 1) * 512],
                             start=True, stop=True)
            nc.vector.tensor_copy(srow[0:1, h * 512:(h + 1) * 512], sp[:])
        # 1/sqrt(sumsq) = exp(-0.5*ln(sumsq)); eps in the reference
        # denominator is negligible (norm >> 1e-5 for random features)
        lnrow = sb_srow.tile([1, csz], F32, tag="lnrow")
        nc.scalar.activation(lnrow[:], srow[:], AF.Ln)
        rrow_r = sb_srow.tile([1, csz], F32R, tag="rrow_r")
        nc.scalar.activation(rrow_r[:], lnrow[:], AF.Exp, scale=-0.5)
        # broadcast 1/s across channel partitions via K=1 matmul
        for h in range(csz // 512):
            pc = pc2 * (csz // 512) + h
            bp = ps1.tile([CQ, 512], F32, tag="ps1")
            nc.tensor.matmul(bp[:], consts.ones_row_cq[:],
                             rrow_r[0:1, h * 512:(h + 1) * 512],
                             start=True, stop=True)
            nc.vector.tensor_tensor(out=feat_tile[0:CQ, pc * 512:(pc + 1) * 512],
                                    in0=f[:, pc * 512:(pc + 1) * 512],
                                    in1=bp[:], op=ALU.mult)
    # feat_tile rows 0:CQ now hold fp32 features; row CQ is the caller's
    # softmax-offset slot (fa) / ones slot (fb)


class _Consts:
    pass


def build():
    nc = bacc.Bacc("TRN2", target_bir_lowering=False, debug=False)
    fa = nc.dram_tensor("fa_roll", [N, C, HW], F32R, kind="ExternalInput").ap()
    fb = nc.dram_tensor("fb_raw", [N, C, HW], F32R, kind="ExternalInput").ap()
    fcT = nc.dram_tensor("fcT", [N, HW, C], F32R, kind="ExternalInput").ap()
    ones_hw = nc.dram_tensor("ones_hw", [1, HW], F32R, kind="ExternalInput").ap()
    WaT = nc.dram_tensor("WaT", [C, CQ], F32, kind="ExternalInput").ap()
    WbT = nc.dram_tensor("WbT", [C, CQ], F32, kind="ExternalInput").ap()
    corr_d = nc.dram_tensor("corr", [N, HW, QS], F32, kind="ExternalOutput").ap()
    warp_d = nc.dram_tensor("warp", [N, C, QS], F32, kind="ExternalOutput").ap()

    with tile.TileContext(nc) as tc:
        import contextlib
        ctx = contextlib.ExitStack()
        with ctx:
            sb_x = ctx.enter_context(tc.tile_pool(name="x", bufs=3))
            sb_y = ctx.enter_context(tc.tile_pool(name="y", bufs=3))
            sb_zf = ctx.enter_context(tc.tile_pool(name="zf", bufs=3))
            sb_scr = ctx.enter_context(tc.tile_pool(name="scr", bufs=3))
            sb_small = ctx.enter_context(tc.tile_pool(name="small", bufs=3))
            sb_srow = ctx.enter_context(tc.tile_pool(name="srow", bufs=2))
            sb_feat = ctx.enter_context(tc.tile_pool(name="feat", bufs=2))
            sb_fafeat = ctx.enter_context(tc.tile_pool(name="fafeat", bufs=2))
            sb_corr = ctx.enter_context(tc.tile_pool(name="corr", bufs=3))
            sb_fc = ctx.enter_context(tc.tile_pool(name="fc", bufs=3))
            sb_warp = ctx.enter_context(tc.tile_pool(name="warp", bufs=1))
            sb_const = ctx.enter_context(tc.tile_pool(name="const", bufs=1))
            ps1 = ctx.enter_context(tc.tile_pool(name="ps1", bufs=2, space="PSUM"))
            psB = ctx.enter_context(tc.tile_pool(name="psB", bufs=2, space="PSUM"))
            psW = ctx.enter_context(tc.tile_pool(name="psW", bufs=2, space="PSUM"))

            # ---- constants ----
            consts = _Consts()
            ident = sb_const.tile([128, 128], F32, tag="ident")
            masks.make_identity(nc, ident[:])
            ones_f32 = sb_const.tile([1, 128], F32, tag="ones_f32")
            nc.vector.memset(ones_f32[:], 1.0)
            ones_col_f32 = sb_const.tile([CQ, 1], F32, tag="ones_col_f32")
            nc.vector.memset(ones_col_f32[:], 1.0)

            ones_cq = sb_const.tile([CQ, 1], F32R, tag="ones_cq")
            nc.vector.tensor_copy(ones_cq[:], ones_col_f32[:])
            ones_row_cq = sb_const.tile([1, CQ], F32R, tag="ones_row_cq")
            nc.vector.tensor_copy(ones_row_cq[:], ones_f32[0:1, 0:CQ])

            consts.ones_cq = ones_cq
            consts.ones_row_cq = ones_row_cq

            # conv weights (transposed on host): [C, CQ] -> two [128, CQ] f32r
            WT_r = {"a": [], "b": []}
            for key, Wd in (("a", WaT), ("b", WbT)):
                for cc in range(2):
                    wt = sb_const.tile([128, CQ], F32, tag=f"w_{key}{cc}")
                    nc.sync.dma_start(wt[:], Wd[cc * 128:(cc + 1) * 128, :])
                    wtr = sb_const.tile([128, CQ], F32R, tag=f"wr_{key}{cc}")
                    nc.vector.tensor_copy(wtr[:], wt[:])
                    WT_r[key].append(wtr)

            
            pools_feat = (sb_x, sb_y, sb_zf, sb_scr, sb_small, sb_srow,
                          consts, ps1)

            for n in range(N):
                # ---- features ----
                fa_feat = sb_fafeat.tile([CQ + 1, QS], F32R, tag="fafeat")
                _feat_pipeline(nc, tc, pools_feat, n, fa, WT_r["a"], fa_feat, QS)
                fb_feat = sb_feat.tile([CQ + 1, HW], F32R, tag="feat")
                _feat_pipeline(nc, tc, pools_feat, n, fb, WT_r["b"], fb_feat, HW)


                # ---- pass A: softmax stats in [q, k] layout ----
                Dstack = sb_small.tile([128, 4], F32, tag="Dstack")
                for j in range(QS // 128):
                    M8 = sb_small.tile([128, 8], F32, tag="M8")
                    S8 = sb_small.tile([128, 8], F32, tag="S8")
                    for kc in range(8):
                        sp = ps1.tile([128, 512], F32, tag="ps1")
                        nc.tensor.matmul(
                            sp[:], fa_feat[0:CQ, j * 128:(j + 1) * 128],
                            fb_feat[0:CQ, kc * 512:(kc + 1) * 512],
                            start=True, stop=True)
                        # flash-style chunk stats straight from PSUM
                        nc.vector.reduce_max(M8[:, kc:kc + 1], sp[:],
                                             axis=mybir.AxisListType.X)
                        biasc = sb_small.tile([128, 1], F32, tag="biasc")
                        nc.vector.tensor_scalar_mul(biasc[:], M8[:, kc:kc + 1],
                                                    -ALPHA)
                        scr = sb_scr.tile([128, 512], BF16, tag="scr")
                        nc.scalar.activation(scr[:], sp[:], AF.Exp,
                                             bias=biasc[:], scale=ALPHA,
                                             accum_out=S8[:, kc:kc + 1])
                    M = sb_small.tile([128, 1], F32, tag="M")
                    nc.vector.reduce_max(M[:], M8[:], axis=mybir.AxisListType.X)
                    # S = sum_c S8_c * exp(100*(M8_c - M))
                    dlt = sb_small.tile([128, 8], F32, tag="dlt")
                    nc.vector.tensor_scalar(out=dlt[:], in0=M8[:], scalar1=M[:],
                                            scalar2=None, op0=ALU.subtract)
                    expd = sb_small.tile([128, 8], F32, tag="expd")
                    nc.scalar.activation(expd[:], dlt[:], AF.Exp, scale=ALPHA)
                    Sw = sb_small.tile([128, 8], F32, tag="Sw")
                    nc.vector.tensor_tensor(out=Sw[:], in0=S8[:], in1=expd[:],
                                            op=ALU.mult)
                    S = sb_small.tile([128, 1], F32, tag="S")
                    nc.vector.reduce_sum(S[:], Sw[:], axis=mybir.AxisListType.X)
                    lnS = sb_small.tile([128, 1], F32, tag="lnS")
                    nc.scalar.activation(lnS[:], S[:], AF.Ln)
                    # aug = -M - lnS/100  (so psum = dot - D/100, D = 100M + lnS)
                    nc.vector.tensor_scalar(out=Dstack[:, j:j + 1], in0=lnS[:],
                                            scalar1=-1.0 / ALPHA, scalar2=None,
                                            op0=ALU.mult)
                    nc.vector.tensor_sub(Dstack[:, j:j + 1], Dstack[:, j:j + 1],
                                         M[:])
                # fb~ row CQ = 1.0, DMA'd from host-provided f32r ones
                nc.sync.dma_start(fb_feat[CQ:CQ + 1, :], ones_hw)
                # transpose Dstack [128, 4] -> [4, 128], stage to SBUF, then
                # assemble the [1, 512] offset row on partition 0 via DMA
                tp = ps1.tile([128, 512], F32, tag="ps1")
                nc.tensor.transpose(tp[0:4, 0:128], Dstack[:], ident[:])
                dstage = sb_small.tile([4, 128], F32, tag="dstage")
                nc.vector.tensor_copy(dstage[:], tp[0:4, 0:128])
                for j in range(QS // 128):
                    nc.sync.dma_start(fa_feat[CQ:CQ + 1, j * 128:(j + 1) * 128],
                                      dstage[j:j + 1, :].bitcast(F32R))

                # ---- pass B + warp ----
                wps = [psW.tile([128, 512], F32, tag="psW", name=f"wps{ct}")
                       for ct in range(2)]
                for g in range(NKT // KGRP):           # 8 groups of 4 k-tiles
                    corr_sb = sb_corr.tile([128, KGRP, 512], F32R, tag="corr")
                    for half in range(2):              # pairs of k-tiles
                        cp = psB.tile([128, 1024], F32, tag="psB")
                        for u in range(2):
                            kt = g * KGRP + half * 2 + u
                            nc.tensor.matmul(
                                cp[:, u * 512:(u + 1) * 512],
                                fb_feat[0:CQ + 1, kt * 128:(kt + 1) * 128],
                                fa_feat[0:CQ + 1, 0:QS],
                                start=True, stop=True)
                        nc.scalar.activation(
                            corr_sb[:, half * 2:half * 2 + 2, :], cp[:],
                            AF.Exp, scale=ALPHA)
                    for u in range(KGRP):
                        kt = g * KGRP + u
                        fctr = sb_fc.tile([128, C], F32R, tag="fc")
                        nc.sync.dma_start(fctr[:],
                                          fcT[n, kt * 128:(kt + 1) * 128, :])
                        for ct in range(2):
                            nc.tensor.matmul(
                                wps[ct][:], fctr[:, ct * 128:(ct + 1) * 128],
                                corr_sb[:, u, :],
                                start=(kt == 0), stop=(kt == NKT - 1))
                    nc.sync.dma_start(
                        corr_d[n, g * KGRP * 128:(g + 1) * KGRP * 128, :]
                        .rearrange("(j p) q -> p j q", p=128),
                        corr_sb[:].bitcast(F32))
                warp_sb = sb_warp.tile([128, 2, 512], F32, tag="warp")
                for ct in range(2):
                    nc.vector.tensor_copy(warp_sb[:, ct, :], wps[ct][:])
                nc.sync.dma_start(
                    warp_d[n].rearrange("(ct p) q -> p ct q", p=128),
                    warp_sb[:])

    nc.compile()
    return nc


_NC_CACHE = None


def _get_nc():
    global _NC_CACHE
    if _NC_CACHE is None:
        _NC_CACHE = build()
    return _NC_CACHE


def make_in_maps(fa_raw, fb_raw, fc_raw, Wa, ba, Wb, bb):
    """Host-side marshalling. ba/bb provably cancel in instance norm."""
    fa2 = np.ascontiguousarray(fa_raw.reshape(N, C, HW), dtype=np.float32)
    fb2 = np.ascontiguousarray(fb_raw.reshape(N, C, HW), dtype=np.float32)
    fcT = np.ascontiguousarray(
        fc_raw.reshape(N, C, HW).transpose(0, 2, 1), dtype=np.float32)
    WaT = np.ascontiguousarray(Wa.T, dtype=np.float32)
    WbT = np.ascontiguousarray(Wb.T, dtype=np.float32)
    ones_hw = np.ones((1, HW), dtype=np.float32)
    in_maps = []
    for core in range(NCORES):
        fa_roll = np.ascontiguousarray(np.roll(fa2, -core * QS, axis=2))
        in_maps.append(dict(fa_roll=fa_roll, fb_raw=fb2, fcT=fcT,
                            WaT=WaT, WbT=WbT, ones_hw=ones_hw))
    return in_maps


LAST_RESULTS = None


def kernel(fa_raw, fb_raw, fc_raw, Wa, ba, Wb, bb):
    global LAST_RESULTS
    nc = _get_nc()
    in_maps = make_in_maps(fa_raw, fb_raw, fc_raw, Wa, ba, Wb, bb)
    res = run_bass_kernel_spmd(nc, in_maps, core_ids=list(range(NCORES)))
    LAST_RESULTS = res
    corr = np.concatenate([res.results[c]["corr"] for c in range(NCORES)],
                          axis=2)
    warp = np.concatenate([res.results[c]["warp"] for c in range(NCORES)],
                          axis=2)
    return warp.reshape(N, C, 64, 64), corr


# revision 31
# speedup vs baseline: 1.9991x; 1.3823x over previous
"""Trainium2 Bass kernel for nn_Attention_47605417508944.

Computes (warp, corr_ab_T) of the reference cross-attention module on 8
NeuronCores, sequence-parallel over the query (fa) axis: each core owns a
512-column shard of the 4096 query positions for all 4 batches.

Host-side marshalling (data movement only):
  - fa_raw is rolled per-core so the core's shard lands at columns 0:512
    (instance-norm / spatial-mean stats are permutation invariant).
  - fc_raw is passed transposed ([n, hw, C]) so warp-matmul weights load
    with unit-stride DMA.
  - Wa/Wb passed transposed ([C, Cq]) to serve directly as conv lhsT.

Math notes:
  - softmax over k handled via an augmented contraction row: the corr-layout
    energy matmul contracts over 65 rows where row 64 of fa~ carries
    -(100*M_q + ln S_q)/100 and row 64 of fb~ is 1.0, so PSUM holds
    dot - D/100 and ACT computes exp(100*psum) = softmax numerator already
    normalized by sum.
  - float32r matmuls (full PE rate, ~1.7e-4 rel err measured on HW).
"""

import numpy as np

import concourse.bacc as bacc
import concourse.tile as tile
from concourse import hw_specs, mybir, masks

# Route all activation-table loads to the one set containing BOTH exp and
# ln: the default chooser alternates exp_and_others / natural_log, paying a
# ~1.3us ACT_TABLE_LOAD per switch (120 loads/kernel measured). Neutering
# the other sets (positions preserved, so set ids stay aligned with
# act_info.json) makes every activation resolve to the combined set.
_orig_get_act_tables = hw_specs.get_activation_tables


def _single_set_tables(arch):
    tabs = dict(_orig_get_act_tables(arch))
    keep = "natural_log_exp_and_others"
    if keep in tabs:
        return {n: (s if n == keep else set()) for n, s in tabs.items()}
    return tabs


bacc.get_activation_tables = _single_set_tables
from concourse.bass_utils import run_bass_kernel_spmd
import concourse.bass as bass

F32 = mybir.dt.float32
F32R = mybir.dt.float32r
BF16 = mybir.dt.bfloat16
AF = mybir.ActivationFunctionType
ALU = mybir.AluOpType

N, C, CQ, HW = 4, 256, 64, 4096
NCORES = 8
QS = HW // NCORES          # 512 query columns per core
NKT = HW // 128            # 32 k-tiles of 128
KGRP = 4                   # k-tiles per corr DMA group
ALPHA = 100.0
EPS = 1e-5


def _feat_pipeline(nc, tc, pools, n, raw_dram, WT_r, feat_tile, cols):
    """Emit feat(x) = L2normalize(center(lrelu(instnorm(W@x)))) for batch n.

    Writes float32r feature rows into feat_tile[0:64, 0:cols].
    cols = QS for fa (shard only), HW for fb (full).
    """
    sb_x, sb_y, sb_zf, sb_scr, sb_small, sb_srow, consts, ps1, psC = pools

    # ---- conv: y[cq, pos] = W.T @ x, f32r matmuls, 8 pos-chunks ----
    y = sb_y.tile([CQ, HW], F32, tag="y")
    ysum8 = sb_small.tile([CQ, 8], F32, tag="ysum8")
    ysq8 = sb_small.tile([CQ, 8], F32, tag="ysq8")
    for pc in range(8):
        yp = psC.tile([CQ, 512], F32, tag="psC")
        for cc in range(2):
            xr = sb_x.tile([128, 512], F32R, tag="x")
            nc.sync.dma_start(xr[:], raw_dram[n, cc * 128:(cc + 1) * 128,
                                              pc * 512:(pc + 1) * 512])
            nc.tensor.matmul(yp[:], WT_r[cc][:], xr[:],
                             start=(cc == 0), stop=(cc == 1))
        # copy psum->sbuf + row-sum accumulation (for spatial mean)
        nc.vector.tensor_scalar(
            out=y[:, pc * 512:(pc + 1) * 512], in0=yp[:], scalar1=0.0,
            scalar2=0.0, op0=ALU.add, op1=ALU.add,
            accum_out=ysum8[:, pc:pc + 1])
        # chunked y^2 with per-chunk sums (pipelines behind the copy)
        ysq = sb_scr.tile([128, 512], BF16, tag="scr")
        nc.vector.scalar_tensor_tensor(
            out=ysq[0:CQ, :], in0=y[:, pc * 512:(pc + 1) * 512], scalar=1.0,
            in1=y[:, pc * 512:(pc + 1) * 512], op0=ALU.mult, op1=ALU.mult,
            accum_out=ysq8[:, pc:pc + 1])

    # ---- instance norm stats ----
    sumsq = sb_small.tile([CQ, 1], F32, tag="sumsq")
    nc.vector.reduce_sum(sumsq[:], ysq8[:], axis=mybir.AxisListType.X)
    ysum = sb_small.tile([CQ, 1], F32, tag="ysum")
    nc.vector.reduce_sum(ysum[:], ysum8[:], axis=mybir.AxisListType.X)
    m = sb_small.tile([CQ, 1], F32, tag="m")
    nc.vector.tensor_scalar_mul(m[:], ysum[:], 1.0 / HW)
    var = sb_small.tile([CQ, 1], F32, tag="var")
    msq = sb_small.tile([CQ, 1], F32, tag="msq")
    nc.vector.tensor_tensor(out=msq[:], in0=m[:], in1=m[:], op=ALU.mult)
    # var = sumsq/HW - m^2 + eps
    nc.vector.tensor_scalar(out=var[:], in0=sumsq[:], scalar1=1.0 / HW,
                            scalar2=None, op0=ALU.mult)
    nc.vector.tensor_sub(var[:], var[:], msq[:])
    nc.vector.tensor_scalar_add(var[:], var[:], EPS)
    # rstd = 1/sqrt(var) via exp/ln (stays in the exp+ln ACT table set)
    lnv = sb_small.tile([CQ, 1], F32, tag="lnv")
    nc.scalar.activation(lnv[:], var[:], AF.Ln)
    rstd = sb_small.tile([CQ, 1], F32, tag="rstd")
    nc.scalar.activation(rstd[:], lnv[:], AF.Exp, scale=-0.5)
    nbias = sb_small.tile([CQ, 1], F32, tag="nbias")
    nc.vector.tensor_tensor(out=nbias[:], in0=m[:], in1=rstd[:], op=ALU.mult)
    nc.vector.tensor_scalar_mul(nbias[:], nbias[:], -1.0)

    # ---- lrelu((y-m)*rstd) = 0.6*t + 0.4*|t|  (t = y*rstd + nbias) ----
    # (decomposed; Lrelu is not CoreSim-checkable)
    t = sb_zf.tile([CQ, HW], F32, tag="zf")
    nc.vector.tensor_scalar(out=t[:], in0=y[:], scalar1=rstd[:],
                            scalar2=nbias[:], op0=ALU.mult, op1=ALU.add)
    z = sb_zf.tile([CQ, HW], F32, tag="zf")
    zsum = sb_small.tile([CQ, 1], F32, tag="zsum")
    nc.vector.scalar_tensor_tensor(out=z[:], in0=t[:], scalar=0.2,
                                   in1=t[:], op0=ALU.mult, op1=ALU.max,
                                   accum_out=zsum[:])
    m2 = sb_small.tile([CQ, 1], F32, tag="m2")
    nc.vector.tensor_scalar_mul(m2[:], zsum[:], -1.0 / HW)

    # ---- center (shard cols only) + channel-L2 normalize ----
    f = sb_zf.tile([CQ, cols], F32, tag="zf")
    nc.vector.tensor_scalar(out=f[:], in0=z[:, 0:cols], scalar1=m2[:],
                            scalar2=None, op0=ALU.add)
    # need f^2 in f32r for the ones-matmul
    fsqr = sb_zf.tile([CQ, cols], F32R, tag="zf")
    nc.vector.scalar_tensor_tensor(out=fsqr[:], in0=f[:], scalar=1.0,
                                   in1=f[:], op0=ALU.mult, op1=ALU.mult)
    # per-position channel L2 norm + broadcast multiply, in 1024-col chunks
    # (row tiles kept small: a [1, N] tile reserves N*4 bytes on every
    #  partition's free-address space)
    csz = min(512, cols)
    for pc2 in range(cols // csz):
        srow = sb_srow.tile([1, csz], F32, tag="srow")
        for h in range(csz // 512):
            pc = pc2 * (csz // 512) + h
            sp = ps1.tile([1, 512], F32, tag="ps1")
            nc.tensor.matmul(sp[:], consts.ones_cq[:],
                             fsqr[:, pc * 512:(pc + 1) * 512],
                             start=True, stop=True)
            nc.vector.tensor_copy(srow[0:1, h * 512:(h + 1) * 512], sp[:])
        # 1/sqrt(sumsq) = exp(-0.5*ln(sumsq)); eps in the reference
        # denominator is negligible (norm >> 1e-5 for random features)
        lnrow = sb_srow.tile([1, csz], F32, tag="lnrow")
        nc.scalar.activation(lnrow[:], srow[:], AF.Ln)
        rrow_r = sb_srow.tile([1, csz], F32R, tag="rrow_r")
        nc.scalar.activation(rrow_r[:], lnrow[:], AF.Exp, scale=-0.5)
        # broadcast 1/s across channel partitions via K=1 matmul
        for h in range(csz // 512):
            pc = pc2 * (csz // 512) + h
            bp = ps1.tile([CQ, 512], F32, tag="ps1")
            nc.tensor.matmul(bp[:], consts.ones_row_cq[:],
                             rrow_r[0:1, h * 512:(h + 1) * 512],
                             start=True, stop=True)
            nc.vector.tensor_tensor(out=feat_tile[0:CQ, pc * 512:(pc + 1) * 512],
                                    in0=f[:, pc * 512:(pc + 1) * 512],
                                    in1=bp[:], op=ALU.mult)
    # feat_tile rows 0:CQ now hold fp32 features; row CQ is the caller's
    # softmax-offset slot (fa) / ones slot (fb)


class _Consts:
    pass


def build():
    nc = bacc.Bacc("TRN2", target_bir_lowering=False, debug=False)
    fa = nc.dram_tensor("fa_roll", [N, C, HW], F32R, kind="ExternalInput").ap()
    fb = nc.dram_tensor("fb_raw", [N, C, HW], F32R, kind="ExternalInput").ap()
    fcT = nc.dram_tensor("fcT", [N, HW, C], F32R, kind="ExternalInput").ap()
    ones_hw = nc.dram_tensor("ones_hw", [1, HW], F32R, kind="ExternalInput").ap()
    WaT = nc.dram_tensor("WaT", [C, CQ], F32, kind="ExternalInput").ap()
    WbT = nc.dram_tensor("WbT", [C, CQ], F32, kind="ExternalInput").ap()
    corr_d = nc.dram_tensor("corr", [N, HW, QS], F32, kind="ExternalOutput").ap()
    warp_d = nc.dram_tensor("warp", [N, C, QS], F32, kind="ExternalOutput").ap()

    with tile.TileContext(nc) as tc:
        import contextlib
        ctx = contextlib.ExitStack()
        with ctx:
            sb_x = ctx.enter_context(tc.tile_pool(name="x", bufs=3))
            sb_y = ctx.enter_context(tc.tile_pool(name="y", bufs=3))
            sb_zf = ctx.enter_context(tc.tile_pool(name="zf", bufs=3))
            sb_scr = ctx.enter_context(tc.tile_pool(name="scr", bufs=3))
            sb_small = ctx.enter_context(tc.tile_pool(name="small", bufs=3))
            sb_srow = ctx.enter_context(tc.tile_pool(name="srow", bufs=2))
            sb_feat = ctx.enter_context(tc.tile_pool(name="feat", bufs=2))
            sb_fafeat = ctx.enter_context(tc.tile_pool(name="fafeat", bufs=2))
            sb_corr = ctx.enter_context(tc.tile_pool(name="corr", bufs=3))
            sb_fc = ctx.enter_context(tc.tile_pool(name="fc", bufs=3))
            sb_warp = ctx.enter_context(tc.tile_pool(name="warp", bufs=1))
            sb_const = ctx.enter_context(tc.tile_pool(name="const", bufs=1))
            ps1 = ctx.enter_context(tc.tile_pool(name="ps1", bufs=2, space="PSUM"))
            psC = ctx.enter_context(tc.tile_pool(name="psC", bufs=2, space="PSUM"))
            psB = ctx.enter_context(tc.tile_pool(name="psB", bufs=2, space="PSUM"))
            psW = ctx.enter_context(tc.tile_pool(name="psW", bufs=2, space="PSUM"))

            # ---- constants ----
            consts = _Consts()
            ident = sb_const.tile([128, 128], F32, tag="ident")
            masks.make_identity(nc, ident[:])
            ones_f32 = sb_const.tile([1, 128], F32, tag="ones_f32")
            nc.vector.memset(ones_f32[:], 1.0)
            ones_col_f32 = sb_const.tile([CQ, 1], F32, tag="ones_col_f32")
            nc.vector.memset(ones_col_f32[:], 1.0)
            neg100 = sb_const.tile([128, 1], F32, tag="neg100")
            nc.vector.memset(neg100[:], -ALPHA)

            ones_cq = sb_const.tile([CQ, 1], F32R, tag="ones_cq")
            nc.vector.tensor_copy(ones_cq[:], ones_col_f32[:])
            ones_row_cq = sb_const.tile([1, CQ], F32R, tag="ones_row_cq")
            nc.vector.tensor_copy(ones_row_cq[:], ones_f32[0:1, 0:CQ])

            consts.ones_cq = ones_cq
            consts.ones_row_cq = ones_row_cq

            # conv weights (transposed on host): [C, CQ] -> two [128, CQ] f32r
            WT_r = {"a": [], "b": []}
            for key, Wd in (("a", WaT), ("b", WbT)):
                for cc in range(2):
                    wt = sb_const.tile([128, CQ], F32, tag=f"w_{key}{cc}")
                    nc.sync.dma_start(wt[:], Wd[cc * 128:(cc + 1) * 128, :])
                    wtr = sb_const.tile([128, CQ], F32R, tag=f"wr_{key}{cc}")
                    nc.vector.tensor_copy(wtr[:], wt[:])
                    WT_r[key].append(wtr)

            
            pools_feat = (sb_x, sb_y, sb_zf, sb_scr, sb_small, sb_srow,
                          consts, ps1, psC)

            for n in range(N):
                # ---- features ----
                fa_feat = sb_fafeat.tile([CQ + 1, QS], F32R, tag="fafeat")
                _feat_pipeline(nc, tc, pools_feat, n, fa, WT_r["a"], fa_feat, QS)
                fb_feat = sb_feat.tile([CQ + 1, HW], F32R, tag="feat")
                _feat_pipeline(nc, tc, pools_feat, n, fb, WT_r["b"], fb_feat, HW)


                # ---- pass A: softmax stats in [q, k] layout ----
                Dstack = sb_small.tile([128, 4], F32, tag="Dstack")
                for j in range(QS // 128):
                    M8 = sb_small.tile([128, 8], F32, tag="M8")
                    S8 = sb_small.tile([128, 8], F32, tag="S8")
                    for kc in range(8):
                        sp = ps1.tile([128, 512], F32, tag="ps1")
                        nc.tensor.matmul(
                            sp[:], fa_feat[0:CQ, j * 128:(j + 1) * 128],
                            fb_feat[0:CQ, kc * 512:(kc + 1) * 512],
                            start=True, stop=True)
                        # flash-style chunk stats; per-chunk max keeps the
                        # HW exp argument in range (<= 0)
                        nc.vector.reduce_max(M8[:, kc:kc + 1], sp[:],
                                             axis=mybir.AxisListType.X)
                        biasc = sb_small.tile([128, 1], F32, tag="biasc")
                        nc.vector.tensor_scalar_mul(biasc[:], M8[:, kc:kc + 1],
                                                    -ALPHA)
                        scr = sb_scr.tile([128, 512], BF16, tag="scr")
                        nc.scalar.activation(scr[:], sp[:], AF.Exp,
                                             bias=biasc[:], scale=ALPHA,
                                             accum_out=S8[:, kc:kc + 1])
                    M = sb_small.tile([128, 1], F32, tag="M")
                    nc.vector.reduce_max(M[:], M8[:], axis=mybir.AxisListType.X)
                    dlt = sb_small.tile([128, 8], F32, tag="dlt")
                    nc.vector.tensor_scalar(out=dlt[:], in0=M8[:], scalar1=M[:],
                                            scalar2=None, op0=ALU.subtract)
                    expd = sb_small.tile([128, 8], F32, tag="expd")
                    nc.scalar.activation(expd[:], dlt[:], AF.Exp, scale=ALPHA)
                    Sw = sb_small.tile([128, 8], F32, tag="Sw")
                    nc.vector.tensor_tensor(out=Sw[:], in0=S8[:], in1=expd[:],
                                            op=ALU.mult)
                    S = sb_small.tile([128, 1], F32, tag="S")
                    nc.vector.reduce_sum(S[:], Sw[:], axis=mybir.AxisListType.X)
                    lnS = sb_small.tile([128, 1], F32, tag="lnS")
                    nc.scalar.activation(lnS[:], S[:], AF.Ln)
                    # aug = -M - lnS/100 (psum = dot - D/100, D = 100M + lnS)
                    nc.vector.tensor_scalar(out=Dstack[:, j:j + 1], in0=lnS[:],
                                            scalar1=-1.0 / ALPHA, scalar2=None,
                                            op0=ALU.mult)
                    nc.vector.tensor_sub(Dstack[:, j:j + 1],
                                         Dstack[:, j:j + 1], M[:])
                # fb~ row CQ = 1.0, DMA'd from host-provided f32r ones
                nc.sync.dma_start(fb_feat[CQ:CQ + 1, :], ones_hw)
                # transpose Dstack [128, 4] -> [4, 128], stage to SBUF, then
                # assemble the [1, 512] offset row on partition 0 via DMA
                tp = ps1.tile([128, 512], F32, tag="ps1")
                nc.tensor.transpose(tp[0:4, 0:128], Dstack[:], ident[:])
                dstage = sb_small.tile([4, 128], F32, tag="dstage")
                nc.vector.tensor_copy(dstage[:], tp[0:4, 0:128])
                for j in range(QS // 128):
                    nc.sync.dma_start(fa_feat[CQ:CQ + 1, j * 128:(j + 1) * 128],
                                      dstage[j:j + 1, :].bitcast(F32R))

                # ---- pass B + warp ----
                wps = [psW.tile([128, 512], F32, tag="psW", name=f"wps{ct}")
                       for ct in range(2)]
                for g in range(NKT // KGRP):           # 8 groups of 4 k-tiles
                    corr_sb = sb_corr.tile([128, KGRP, 512], F32R, tag="corr")
                    for u in range(KGRP):
                        kt = g * KGRP + u
                        cp = psB.tile([128, 512], F32, tag="psB")
                        nc.tensor.matmul(
                            cp[:],
                            fb_feat[0:CQ + 1, kt * 128:(kt + 1) * 128],
                            fa_feat[0:CQ + 1, 0:QS],
                            start=True, stop=True)
                        nc.scalar.activation(
                            corr_sb[:, u, :], cp[:], AF.Exp, scale=ALPHA)
                    for u in range(KGRP):
                        kt = g * KGRP + u
                        fctr = sb_fc.tile([128, C], F32R, tag="fc")
                        nc.sync.dma_start(fctr[:],
                                          fcT[n, kt * 128:(kt + 1) * 128, :])
                        for ct in range(2):
                            nc.tensor.matmul(
                                wps[ct][:], fctr[:, ct * 128:(ct + 1) * 128],
                                corr_sb[:, u, :],
                                start=(kt == 0), stop=(kt == NKT - 1))
                    nc.sync.dma_start(
                        corr_d[n, g * KGRP * 128:(g + 1) * KGRP * 128, :]
                        .rearrange("(j p) q -> p j q", p=128),
                        corr_sb[:].bitcast(F32))
                warp_sb = sb_warp.tile([128, 2, 512], F32, tag="warp")
                for ct in range(2):
                    nc.vector.tensor_copy(warp_sb[:, ct, :], wps[ct][:])
                nc.sync.dma_start(
                    warp_d[n].rearrange("(ct p) q -> p ct q", p=128),
                    warp_sb[:])

    nc.compile()
    return nc


_NC_CACHE = None


def _get_nc():
    global _NC_CACHE
    if _NC_CACHE is None:
        _NC_CACHE = build()
    return _NC_CACHE


def make_in_maps(fa_raw, fb_raw, fc_raw, Wa, ba, Wb, bb):
    """Host-side marshalling. ba/bb provably cancel in instance norm."""
    fa2 = np.ascontiguousarray(fa_raw.reshape(N, C, HW), dtype=np.float32)
    fb2 = np.ascontiguousarray(fb_raw.reshape(N, C, HW), dtype=np.float32)
    fcT = np.ascontiguousarray(
        fc_raw.reshape(N, C, HW).transpose(0, 2, 1), dtype=np.float32)
    WaT = np.ascontiguousarray(Wa.T, dtype=np.float32)
    WbT = np.ascontiguousarray(Wb.T, dtype=np.float32)
    ones_hw = np.ones((1, HW), dtype=np.float32)
    in_maps = []
    for core in range(NCORES):
        fa_roll = np.ascontiguousarray(np.roll(fa2, -core * QS, axis=2))
        in_maps.append(dict(fa_roll=fa_roll, fb_raw=fb2, fcT=fcT,
                            WaT=WaT, WbT=WbT, ones_hw=ones_hw))
    return in_maps


LAST_RESULTS = None


def kernel(fa_raw, fb_raw, fc_raw, Wa, ba, Wb, bb):
    global LAST_RESULTS
    nc = _get_nc()
    in_maps = make_in_maps(fa_raw, fb_raw, fc_raw, Wa, ba, Wb, bb)
    res = run_bass_kernel_spmd(nc, in_maps, core_ids=list(range(NCORES)))
    LAST_RESULTS = res
    corr = np.concatenate([res.results[c]["corr"] for c in range(NCORES)],
                          axis=2)
    warp = np.concatenate([res.results[c]["warp"] for c in range(NCORES)],
                          axis=2)
    return warp.reshape(N, C, 64, 64), corr


# revision 32
# speedup vs baseline: 2.1160x; 1.0585x over previous
"""Trainium2 Bass kernel for nn_Attention_47605417508944.

Computes (warp, corr_ab_T) of the reference cross-attention module on 8
NeuronCores, sequence-parallel over the query (fa) axis: each core owns a
512-column shard of the 4096 query positions for all 4 batches.

Host-side marshalling (data movement only):
  - fa_raw is rolled per-core so the core's shard lands at columns 0:512
    (instance-norm / spatial-mean stats are permutation invariant).
  - fc_raw is passed transposed ([n, hw, C]) so warp-matmul weights load
    with unit-stride DMA.
  - Wa/Wb passed transposed ([C, Cq]) to serve directly as conv lhsT.

Math notes:
  - softmax over k handled via an augmented contraction row: the corr-layout
    energy matmul contracts over 65 rows where row 64 of fa~ carries
    -(100*M_q + ln S_q)/100 and row 64 of fb~ is 1.0, so PSUM holds
    dot - D/100 and ACT computes exp(100*psum) = softmax numerator already
    normalized by sum.
  - float32r matmuls (full PE rate, ~1.7e-4 rel err measured on HW).
"""

import numpy as np

import concourse.bacc as bacc
import concourse.tile as tile
from concourse import hw_specs, mybir, masks

# Route all activation-table loads to the one set containing BOTH exp and
# ln: the default chooser alternates exp_and_others / natural_log, paying a
# ~1.3us ACT_TABLE_LOAD per switch (120 loads/kernel measured). Neutering
# the other sets (positions preserved, so set ids stay aligned with
# act_info.json) makes every activation resolve to the combined set.
_orig_get_act_tables = hw_specs.get_activation_tables


def _single_set_tables(arch):
    tabs = dict(_orig_get_act_tables(arch))
    keep = "natural_log_exp_and_others"
    if keep in tabs:
        return {n: (s if n == keep else set()) for n, s in tabs.items()}
    return tabs


bacc.get_activation_tables = _single_set_tables
from concourse.bass_utils import run_bass_kernel_spmd
import concourse.bass as bass

F32 = mybir.dt.float32
F32R = mybir.dt.float32r
BF16 = mybir.dt.bfloat16
AF = mybir.ActivationFunctionType
ALU = mybir.AluOpType

N, C, CQ, HW = 4, 256, 64, 4096
NCORES = 8
QS = HW // NCORES          # 512 query columns per core
NKT = HW // 128            # 32 k-tiles of 128
KGRP = 4                   # k-tiles per corr DMA group
ALPHA = 100.0
EPS = 1e-5


def _feat_pipeline(nc, tc, pools, n, raw_dram, WT_r, feat_tile, cols):
    """Emit feat(x) = L2normalize(center(lrelu(instnorm(W@x)))) for batch n.

    Writes float32r feature rows into feat_tile[0:64, 0:cols].
    cols = QS for fa (shard only), HW for fb (full).
    """
    sb_x, sb_y, sb_zf, sb_scr, sb_small, sb_srow, consts, ps1, psC = pools

    # ---- conv: y[cq, pos] = W.T @ x, f32r matmuls, 8 pos-chunks ----
    y = sb_y.tile([CQ, HW], F32, tag="y")
    ysum8 = sb_small.tile([CQ, 8], F32, tag="ysum8")
    ysq8 = sb_small.tile([CQ, 8], F32, tag="ysq8")
    for pc in range(8):
        yp = psC.tile([CQ, 512], F32, tag="psC")
        for cc in range(2):
            xr = sb_x.tile([128, 512], F32R, tag="x")
            nc.sync.dma_start(xr[:], raw_dram[n, cc * 128:(cc + 1) * 128,
                                              pc * 512:(pc + 1) * 512])
            nc.tensor.matmul(yp[:], WT_r[cc][:], xr[:],
                             start=(cc == 0), stop=(cc == 1))
        # copy psum->sbuf + row-sum accumulation (for spatial mean)
        nc.vector.tensor_scalar(
            out=y[:, pc * 512:(pc + 1) * 512], in0=yp[:], scalar1=0.0,
            scalar2=0.0, op0=ALU.add, op1=ALU.add,
            accum_out=ysum8[:, pc:pc + 1])
        # chunked y^2 with per-chunk sums (pipelines behind the copy)
        ysq = sb_scr.tile([128, 512], BF16, tag="scr")
        nc.vector.scalar_tensor_tensor(
            out=ysq[0:CQ, :], in0=y[:, pc * 512:(pc + 1) * 512], scalar=1.0,
            in1=y[:, pc * 512:(pc + 1) * 512], op0=ALU.mult, op1=ALU.mult,
            accum_out=ysq8[:, pc:pc + 1])

    # ---- instance norm stats ----
    sumsq = sb_small.tile([CQ, 1], F32, tag="sumsq")
    nc.vector.reduce_sum(sumsq[:], ysq8[:], axis=mybir.AxisListType.X)
    ysum = sb_small.tile([CQ, 1], F32, tag="ysum")
    nc.vector.reduce_sum(ysum[:], ysum8[:], axis=mybir.AxisListType.X)
    m = sb_small.tile([CQ, 1], F32, tag="m")
    nc.vector.tensor_scalar_mul(m[:], ysum[:], 1.0 / HW)
    var = sb_small.tile([CQ, 1], F32, tag="var")
    msq = sb_small.tile([CQ, 1], F32, tag="msq")
    nc.vector.tensor_tensor(out=msq[:], in0=m[:], in1=m[:], op=ALU.mult)
    # var = sumsq/HW - m^2 + eps
    nc.vector.tensor_scalar(out=var[:], in0=sumsq[:], scalar1=1.0 / HW,
                            scalar2=None, op0=ALU.mult)
    nc.vector.tensor_sub(var[:], var[:], msq[:])
    nc.vector.tensor_scalar_add(var[:], var[:], EPS)
    # rstd = 1/sqrt(var) via exp/ln (stays in the exp+ln ACT table set)
    lnv = sb_small.tile([CQ, 1], F32, tag="lnv")
    nc.scalar.activation(lnv[:], var[:], AF.Ln)
    rstd = sb_small.tile([CQ, 1], F32, tag="rstd")
    nc.scalar.activation(rstd[:], lnv[:], AF.Exp, scale=-0.5)
    nbias = sb_small.tile([CQ, 1], F32, tag="nbias")
    nc.vector.tensor_tensor(out=nbias[:], in0=m[:], in1=rstd[:], op=ALU.mult)
    nc.vector.tensor_scalar_mul(nbias[:], nbias[:], -1.0)

    # ---- lrelu((y-m)*rstd) = 0.6*t + 0.4*|t|  (t = y*rstd + nbias) ----
    # (decomposed; Lrelu is not CoreSim-checkable)
    t = sb_zf.tile([CQ, HW], F32, tag="zf")
    nc.vector.tensor_scalar(out=t[:], in0=y[:], scalar1=rstd[:],
                            scalar2=nbias[:], op0=ALU.mult, op1=ALU.add)
    z = sb_zf.tile([CQ, HW], F32, tag="zf")
    zsum = sb_small.tile([CQ, 1], F32, tag="zsum")
    nc.vector.scalar_tensor_tensor(out=z[:], in0=t[:], scalar=0.2,
                                   in1=t[:], op0=ALU.mult, op1=ALU.max,
                                   accum_out=zsum[:])
    m2 = sb_small.tile([CQ, 1], F32, tag="m2")
    nc.vector.tensor_scalar_mul(m2[:], zsum[:], -1.0 / HW)

    # ---- center (shard cols only) + channel-L2 normalize ----
    f = sb_zf.tile([CQ, cols], F32, tag="zf")
    nc.vector.tensor_scalar(out=f[:], in0=z[:, 0:cols], scalar1=m2[:],
                            scalar2=None, op0=ALU.add)
    # need f^2 in f32r for the ones-matmul
    fsqr = sb_zf.tile([CQ, cols], F32R, tag="zf")
    nc.vector.scalar_tensor_tensor(out=fsqr[:], in0=f[:], scalar=1.0,
                                   in1=f[:], op0=ALU.mult, op1=ALU.mult)
    # per-position channel L2 norm + broadcast multiply, in 1024-col chunks
    # (row tiles kept small: a [1, N] tile reserves N*4 bytes on every
    #  partition's free-address space)
    csz = min(512, cols)
    for pc2 in range(cols // csz):
        srow = sb_srow.tile([1, csz], F32, tag="srow")
        for h in range(csz // 512):
            pc = pc2 * (csz // 512) + h
            sp = ps1.tile([1, 512], F32, tag="ps1")
            nc.tensor.matmul(sp[:], consts.ones_cq[:],
                             fsqr[:, pc * 512:(pc + 1) * 512],
                             start=True, stop=True)
            nc.vector.tensor_copy(srow[0:1, h * 512:(h + 1) * 512], sp[:])
        # 1/sqrt(sumsq) = exp(-0.5*ln(sumsq)); eps in the reference
        # denominator is negligible (norm >> 1e-5 for random features)
        lnrow = sb_srow.tile([1, csz], F32, tag="lnrow")
        nc.scalar.activation(lnrow[:], srow[:], AF.Ln)
        rrow_r = sb_srow.tile([1, csz], F32R, tag="rrow_r")
        nc.scalar.activation(rrow_r[:], lnrow[:], AF.Exp, scale=-0.5)
        # broadcast 1/s across channel partitions via K=1 matmul
        for h in range(csz // 512):
            pc = pc2 * (csz // 512) + h
            bp = ps1.tile([CQ, 512], F32, tag="ps1")
            nc.tensor.matmul(bp[:], consts.ones_row_cq[:],
                             rrow_r[0:1, h * 512:(h + 1) * 512],
                             start=True, stop=True)
            nc.vector.tensor_tensor(out=feat_tile[0:CQ, pc * 512:(pc + 1) * 512],
                                    in0=f[:, pc * 512:(pc + 1) * 512],
                                    in1=bp[:], op=ALU.mult)
    # feat_tile rows 0:CQ now hold fp32 features; row CQ is the caller's
    # softmax-offset slot (fa) / ones slot (fb)


class _Consts:
    pass


def build():
    nc = bacc.Bacc("TRN2", target_bir_lowering=False, debug=False)
    fa = nc.dram_tensor("fa_roll", [N, C, HW], F32R, kind="ExternalInput").ap()
    fb = nc.dram_tensor("fb_raw", [N, C, HW], F32R, kind="ExternalInput").ap()
    fcT = nc.dram_tensor("fcT", [N, HW, C], F32R, kind="ExternalInput").ap()
    ones_hw = nc.dram_tensor("ones_hw", [1, HW], F32R, kind="ExternalInput").ap()
    WaT = nc.dram_tensor("WaT", [C, CQ], F32, kind="ExternalInput").ap()
    WbT = nc.dram_tensor("WbT", [C, CQ], F32, kind="ExternalInput").ap()
    corr_d = nc.dram_tensor("corr", [N, HW, QS], F32, kind="ExternalOutput").ap()
    warp_d = nc.dram_tensor("warp", [N, C, QS], F32, kind="ExternalOutput").ap()

    with tile.TileContext(nc) as tc:
        import contextlib
        ctx = contextlib.ExitStack()
        with ctx:
            sb_x = ctx.enter_context(tc.tile_pool(name="x", bufs=6))
            sb_y = ctx.enter_context(tc.tile_pool(name="y", bufs=3))
            sb_zf = ctx.enter_context(tc.tile_pool(name="zf", bufs=3))
            sb_scr = ctx.enter_context(tc.tile_pool(name="scr", bufs=3))
            sb_small = ctx.enter_context(tc.tile_pool(name="small", bufs=3))
            sb_srow = ctx.enter_context(tc.tile_pool(name="srow", bufs=2))
            sb_feat = ctx.enter_context(tc.tile_pool(name="feat", bufs=2))
            sb_fafeat = ctx.enter_context(tc.tile_pool(name="fafeat", bufs=2))
            sb_corr = ctx.enter_context(tc.tile_pool(name="corr", bufs=3))
            sb_fc = ctx.enter_context(tc.tile_pool(name="fc", bufs=4))
            sb_warp = ctx.enter_context(tc.tile_pool(name="warp", bufs=1))
            sb_const = ctx.enter_context(tc.tile_pool(name="const", bufs=1))
            ps1 = ctx.enter_context(tc.tile_pool(name="ps1", bufs=2, space="PSUM"))
            psC = ctx.enter_context(tc.tile_pool(name="psC", bufs=2, space="PSUM"))
            psB = ctx.enter_context(tc.tile_pool(name="psB", bufs=2, space="PSUM"))
            psW = ctx.enter_context(tc.tile_pool(name="psW", bufs=2, space="PSUM"))

            # ---- constants ----
            consts = _Consts()
            ident = sb_const.tile([128, 128], F32, tag="ident")
            masks.make_identity(nc, ident[:])
            ones_f32 = sb_const.tile([1, 128], F32, tag="ones_f32")
            nc.vector.memset(ones_f32[:], 1.0)
            ones_col_f32 = sb_const.tile([CQ, 1], F32, tag="ones_col_f32")
            nc.vector.memset(ones_col_f32[:], 1.0)
            neg100 = sb_const.tile([128, 1], F32, tag="neg100")
            nc.vector.memset(neg100[:], -ALPHA)

            ones_cq = sb_const.tile([CQ, 1], F32R, tag="ones_cq")
            nc.vector.tensor_copy(ones_cq[:], ones_col_f32[:])
            ones_row_cq = sb_const.tile([1, CQ], F32R, tag="ones_row_cq")
            nc.vector.tensor_copy(ones_row_cq[:], ones_f32[0:1, 0:CQ])

            consts.ones_cq = ones_cq
            consts.ones_row_cq = ones_row_cq

            # conv weights (transposed on host): [C, CQ] -> two [128, CQ] f32r
            WT_r = {"a": [], "b": []}
            for key, Wd in (("a", WaT), ("b", WbT)):
                for cc in range(2):
                    wt = sb_const.tile([128, CQ], F32, tag=f"w_{key}{cc}")
                    nc.sync.dma_start(wt[:], Wd[cc * 128:(cc + 1) * 128, :])
                    wtr = sb_const.tile([128, CQ], F32R, tag=f"wr_{key}{cc}")
                    nc.vector.tensor_copy(wtr[:], wt[:])
                    WT_r[key].append(wtr)

            
            pools_feat = (sb_x, sb_y, sb_zf, sb_scr, sb_small, sb_srow,
                          consts, ps1, psC)

            for n in range(N):
                # ---- features ----
                fa_feat = sb_fafeat.tile([CQ + 1, QS], F32R, tag="fafeat")
                _feat_pipeline(nc, tc, pools_feat, n, fa, WT_r["a"], fa_feat, QS)
                fb_feat = sb_feat.tile([CQ + 1, HW], F32R, tag="feat")
                _feat_pipeline(nc, tc, pools_feat, n, fb, WT_r["b"], fb_feat, HW)


                # ---- pass A: softmax stats in [q, k] layout ----
                Dstack = sb_small.tile([128, 4], F32, tag="Dstack")
                for j in range(QS // 128):
                    M8 = sb_small.tile([128, 8], F32, tag="M8")
                    S8 = sb_small.tile([128, 8], F32, tag="S8")
                    for kc in range(8):
                        sp = ps1.tile([128, 512], F32, tag="ps1")
                        nc.tensor.matmul(
                            sp[:], fa_feat[0:CQ, j * 128:(j + 1) * 128],
                            fb_feat[0:CQ, kc * 512:(kc + 1) * 512],
                            start=True, stop=True)
                        # flash-style chunk stats; per-chunk max keeps the
                        # HW exp argument in range (<= 0)
                        nc.vector.reduce_max(M8[:, kc:kc + 1], sp[:],
                                             axis=mybir.AxisListType.X)
                        biasc = sb_small.tile([128, 1], F32, tag="biasc")
                        nc.vector.tensor_scalar_mul(biasc[:], M8[:, kc:kc + 1],
                                                    -ALPHA)
                        scr = sb_scr.tile([128, 512], BF16, tag="scr")
                        nc.scalar.activation(scr[:], sp[:], AF.Exp,
                                             bias=biasc[:], scale=ALPHA,
                                             accum_out=S8[:, kc:kc + 1])
                    M = sb_small.tile([128, 1], F32, tag="M")
                    nc.vector.reduce_max(M[:], M8[:], axis=mybir.AxisListType.X)
                    dlt = sb_small.tile([128, 8], F32, tag="dlt")
                    nc.vector.tensor_scalar(out=dlt[:], in0=M8[:], scalar1=M[:],
                                            scalar2=None, op0=ALU.subtract)
                    expd = sb_small.tile([128, 8], F32, tag="expd")
                    nc.scalar.activation(expd[:], dlt[:], AF.Exp, scale=ALPHA)
                    Sw = sb_small.tile([128, 8], F32, tag="Sw")
                    nc.vector.tensor_tensor(out=Sw[:], in0=S8[:], in1=expd[:],
                                            op=ALU.mult)
                    S = sb_small.tile([128, 1], F32, tag="S")
                    nc.vector.reduce_sum(S[:], Sw[:], axis=mybir.AxisListType.X)
                    lnS = sb_small.tile([128, 1], F32, tag="lnS")
                    nc.scalar.activation(lnS[:], S[:], AF.Ln)
                    # aug = -M - lnS/100 (psum = dot - D/100, D = 100M + lnS)
                    nc.vector.tensor_scalar(out=Dstack[:, j:j + 1], in0=lnS[:],
                                            scalar1=-1.0 / ALPHA, scalar2=None,
                                            op0=ALU.mult)
                    nc.vector.tensor_sub(Dstack[:, j:j + 1],
                                         Dstack[:, j:j + 1], M[:])
                # fb~ row CQ = 1.0, DMA'd from host-provided f32r ones
                nc.sync.dma_start(fb_feat[CQ:CQ + 1, :], ones_hw)
                # transpose Dstack [128, 4] -> [4, 128], stage to SBUF, then
                # assemble the [1, 512] offset row on partition 0 via DMA
                tp = ps1.tile([128, 512], F32, tag="ps1")
                nc.tensor.transpose(tp[0:4, 0:128], Dstack[:], ident[:])
                dstage = sb_small.tile([4, 128], F32, tag="dstage")
                nc.vector.tensor_copy(dstage[:], tp[0:4, 0:128])
                for j in range(QS // 128):
                    nc.sync.dma_start(fa_feat[CQ:CQ + 1, j * 128:(j + 1) * 128],
                                      dstage[j:j + 1, :].bitcast(F32R))

                # ---- pass B + warp ----
                wps = [psW.tile([128, 512], F32, tag="psW", name=f"wps{ct}")
                       for ct in range(2)]
                for g in range(NKT // KGRP):           # 8 groups of 4 k-tiles
                    corr_sb = sb_corr.tile([128, KGRP, 512], F32R, tag="corr")
                    for u in range(KGRP):
                        kt = g * KGRP + u
                        cp = psB.tile([128, 512], F32, tag="psB")
                        nc.tensor.matmul(
                            cp[:],
                            fb_feat[0:CQ + 1, kt * 128:(kt + 1) * 128],
                            fa_feat[0:CQ + 1, 0:QS],
                            start=True, stop=True)
                        nc.scalar.activation(
                            corr_sb[:, u, :], cp[:], AF.Exp, scale=ALPHA)
                    for u in range(KGRP):
                        kt = g * KGRP + u
                        fctr = sb_fc.tile([128, C], F32R, tag="fc")
                        nc.sync.dma_start(fctr[:],
                                          fcT[n, kt * 128:(kt + 1) * 128, :])
                        for ct in range(2):
                            nc.tensor.matmul(
                                wps[ct][:], fctr[:, ct * 128:(ct + 1) * 128],
                                corr_sb[:, u, :],
                                start=(kt == 0), stop=(kt == NKT - 1))
                    nc.sync.dma_start(
                        corr_d[n, g * KGRP * 128:(g + 1) * KGRP * 128, :]
                        .rearrange("(j p) q -> p j q", p=128),
                        corr_sb[:].bitcast(F32))
                warp_sb = sb_warp.tile([128, 2, 512], F32, tag="warp")
                for ct in range(2):
                    nc.vector.tensor_copy(warp_sb[:, ct, :], wps[ct][:])
                nc.sync.dma_start(
                    warp_d[n].rearrange("(ct p) q -> p ct q", p=128),
                    warp_sb[:])

    nc.compile()
    return nc


_NC_CACHE = None


def _get_nc():
    global _NC_CACHE
    if _NC_CACHE is None:
        _NC_CACHE = build()
    return _NC_CACHE


def make_in_maps(fa_raw, fb_raw, fc_raw, Wa, ba, Wb, bb):
    """Host-side marshalling. ba/bb provably cancel in instance norm."""
    fa2 = np.ascontiguousarray(fa_raw.reshape(N, C, HW), dtype=np.float32)
    fb2 = np.ascontiguousarray(fb_raw.reshape(N, C, HW), dtype=np.float32)
    fcT = np.ascontiguousarray(
        fc_raw.reshape(N, C, HW).transpose(0, 2, 1), dtype=np.float32)
    WaT = np.ascontiguousarray(Wa.T, dtype=np.float32)
    WbT = np.ascontiguousarray(Wb.T, dtype=np.float32)
    ones_hw = np.ones((1, HW), dtype=np.float32)
    in_maps = []
    for core in range(NCORES):
        fa_roll = np.ascontiguousarray(np.roll(fa2, -core * QS, axis=2))
        in_maps.append(dict(fa_roll=fa_roll, fb_raw=fb2, fcT=fcT,
                            WaT=WaT, WbT=WbT, ones_hw=ones_hw))
    return in_maps


LAST_RESULTS = None


def kernel(fa_raw, fb_raw, fc_raw, Wa, ba, Wb, bb):
    global LAST_RESULTS
    nc = _get_nc()
    in_maps = make_in_maps(fa_raw, fb_raw, fc_raw, Wa, ba, Wb, bb)
    res = run_bass_kernel_spmd(nc, in_maps, core_ids=list(range(NCORES)))
    LAST_RESULTS = res
    corr = np.concatenate([res.results[c]["corr"] for c in range(NCORES)],
                          axis=2)
    warp = np.concatenate([res.results[c]["warp"] for c in range(NCORES)],
                          axis=2)
    return warp.reshape(N, C, 64, 64), corr


# revision 33
# speedup vs baseline: 2.1904x; 1.0352x over previous
"""Trainium2 Bass kernel for nn_Attention_47605417508944.

Computes (warp, corr_ab_T) of the reference cross-attention module on 8
NeuronCores, sequence-parallel over the query (fa) axis: each core owns a
512-column shard of the 4096 query positions for all 4 batches.

Host-side marshalling (data movement only):
  - fa_raw is rolled per-core so the core's shard lands at columns 0:512
    (instance-norm / spatial-mean stats are permutation invariant).
  - fc_raw is passed transposed ([n, hw, C]) so warp-matmul weights load
    with unit-stride DMA.
  - Wa/Wb passed transposed ([C, Cq]) to serve directly as conv lhsT.

Math notes:
  - softmax over k handled via an augmented contraction row: the corr-layout
    energy matmul contracts over 65 rows where row 64 of fa~ carries
    -(100*M_q + ln S_q)/100 and row 64 of fb~ is 1.0, so PSUM holds
    dot - D/100 and ACT computes exp(100*psum) = softmax numerator already
    normalized by sum.
  - float32r matmuls (full PE rate, ~1.7e-4 rel err measured on HW).
"""

import numpy as np

import concourse.bacc as bacc
import concourse.tile as tile
from concourse import hw_specs, mybir, masks

# Route all activation-table loads to the one set containing BOTH exp and
# ln: the default chooser alternates exp_and_others / natural_log, paying a
# ~1.3us ACT_TABLE_LOAD per switch (120 loads/kernel measured). Neutering
# the other sets (positions preserved, so set ids stay aligned with
# act_info.json) makes every activation resolve to the combined set.
_orig_get_act_tables = hw_specs.get_activation_tables


def _single_set_tables(arch):
    tabs = dict(_orig_get_act_tables(arch))
    keep = "natural_log_exp_and_others"
    if keep in tabs:
        return {n: (s if n == keep else set()) for n, s in tabs.items()}
    return tabs


bacc.get_activation_tables = _single_set_tables
from concourse.bass_utils import run_bass_kernel_spmd
import concourse.bass as bass

F32 = mybir.dt.float32
F32R = mybir.dt.float32r
BF16 = mybir.dt.bfloat16
AF = mybir.ActivationFunctionType
ALU = mybir.AluOpType

N, C, CQ, HW = 4, 256, 64, 4096
NCORES = 8
QS = HW // NCORES          # 512 query columns per core
NKT = HW // 128            # 32 k-tiles of 128
KGRP = 4                   # k-tiles per corr DMA group
ALPHA = 100.0
EPS = 1e-5


def _feat_pipeline(nc, tc, pools, n, raw_dram, WT_r, feat_tile, cols):
    """Emit feat(x) = L2normalize(center(lrelu(instnorm(W@x)))) for batch n.

    Writes float32r feature rows into feat_tile[0:64, 0:cols].
    cols = QS for fa (shard only), HW for fb (full).
    """
    sb_x, sb_y, sb_zf, sb_scr, sb_small, sb_srow, consts, ps1, psC = pools

    # ---- conv: y[cq, pos] = W.T @ x, f32r matmuls, 8 pos-chunks ----
    y = sb_y.tile([CQ, HW], F32, tag="y")
    ysum8 = sb_small.tile([CQ, 8], F32, tag="ysum8")
    ysq8 = sb_small.tile([CQ, 8], F32, tag="ysq8")
    for pc in range(8):
        yp = psC.tile([CQ, 512], F32, tag="psC")
        for cc in range(2):
            xr = sb_x.tile([128, 512], F32R, tag="x")
            nc.sync.dma_start(xr[:], raw_dram[n, cc * 128:(cc + 1) * 128,
                                              pc * 512:(pc + 1) * 512])
            nc.tensor.matmul(yp[:], WT_r[cc][:], xr[:],
                             start=(cc == 0), stop=(cc == 1))
        # copy psum->sbuf + row-sum accumulation (for spatial mean)
        nc.vector.tensor_scalar(
            out=y[:, pc * 512:(pc + 1) * 512], in0=yp[:], scalar1=0.0,
            scalar2=0.0, op0=ALU.add, op1=ALU.add,
            accum_out=ysum8[:, pc:pc + 1])
        # chunked y^2 with per-chunk sums (pipelines behind the copy)
        ysq = sb_scr.tile([128, 512], BF16, tag="scr")
        nc.vector.scalar_tensor_tensor(
            out=ysq[0:CQ, :], in0=y[:, pc * 512:(pc + 1) * 512], scalar=1.0,
            in1=y[:, pc * 512:(pc + 1) * 512], op0=ALU.mult, op1=ALU.mult,
            accum_out=ysq8[:, pc:pc + 1])

    # ---- instance norm stats ----
    sumsq = sb_small.tile([CQ, 1], F32, tag="sumsq")
    nc.vector.reduce_sum(sumsq[:], ysq8[:], axis=mybir.AxisListType.X)
    ysum = sb_small.tile([CQ, 1], F32, tag="ysum")
    nc.vector.reduce_sum(ysum[:], ysum8[:], axis=mybir.AxisListType.X)
    m = sb_small.tile([CQ, 1], F32, tag="m")
    nc.vector.tensor_scalar_mul(m[:], ysum[:], 1.0 / HW)
    var = sb_small.tile([CQ, 1], F32, tag="var")
    msq = sb_small.tile([CQ, 1], F32, tag="msq")
    nc.vector.tensor_tensor(out=msq[:], in0=m[:], in1=m[:], op=ALU.mult)
    # var = sumsq/HW - m^2 + eps
    nc.vector.tensor_scalar(out=var[:], in0=sumsq[:], scalar1=1.0 / HW,
                            scalar2=None, op0=ALU.mult)
    nc.vector.tensor_sub(var[:], var[:], msq[:])
    nc.vector.tensor_scalar_add(var[:], var[:], EPS)
    # rstd = 1/sqrt(var) via exp/ln (stays in the exp+ln ACT table set)
    lnv = sb_small.tile([CQ, 1], F32, tag="lnv")
    nc.scalar.activation(lnv[:], var[:], AF.Ln)
    rstd = sb_small.tile([CQ, 1], F32, tag="rstd")
    nc.scalar.activation(rstd[:], lnv[:], AF.Exp, scale=-0.5)
    nbias = sb_small.tile([CQ, 1], F32, tag="nbias")
    nc.vector.tensor_tensor(out=nbias[:], in0=m[:], in1=rstd[:], op=ALU.mult)
    nc.vector.tensor_scalar_mul(nbias[:], nbias[:], -1.0)

    # ---- lrelu((y-m)*rstd) = 0.6*t + 0.4*|t|  (t = y*rstd + nbias) ----
    # (decomposed; Lrelu is not CoreSim-checkable)
    t = sb_zf.tile([CQ, HW], F32, tag="zf")
    nc.vector.tensor_scalar(out=t[:], in0=y[:], scalar1=rstd[:],
                            scalar2=nbias[:], op0=ALU.mult, op1=ALU.add)
    z = sb_zf.tile([CQ, HW], F32, tag="zf")
    zsum = sb_small.tile([CQ, 1], F32, tag="zsum")
    nc.vector.scalar_tensor_tensor(out=z[:], in0=t[:], scalar=0.2,
                                   in1=t[:], op0=ALU.mult, op1=ALU.max,
                                   accum_out=zsum[:])
    m2 = sb_small.tile([CQ, 1], F32, tag="m2")
    nc.vector.tensor_scalar_mul(m2[:], zsum[:], -1.0 / HW)

    # ---- center (shard cols only) + channel-L2 normalize ----
    f = sb_zf.tile([CQ, cols], F32, tag="zf")
    nc.vector.tensor_scalar(out=f[:], in0=z[:, 0:cols], scalar1=m2[:],
                            scalar2=None, op0=ALU.add)
    # need f^2 in f32r for the ones-matmul
    fsqr = sb_zf.tile([CQ, cols], F32R, tag="zf")
    nc.vector.scalar_tensor_tensor(out=fsqr[:], in0=f[:], scalar=1.0,
                                   in1=f[:], op0=ALU.mult, op1=ALU.mult)
    # per-position channel L2 norm + broadcast multiply, in 1024-col chunks
    # (row tiles kept small: a [1, N] tile reserves N*4 bytes on every
    #  partition's free-address space)
    csz = min(512, cols)
    for pc2 in range(cols // csz):
        srow = sb_srow.tile([1, csz], F32, tag="srow")
        for h in range(csz // 512):
            pc = pc2 * (csz // 512) + h
            sp = ps1.tile([1, 512], F32, tag="ps1")
            nc.tensor.matmul(sp[:], consts.ones_cq[:],
                             fsqr[:, pc * 512:(pc + 1) * 512],
                             start=True, stop=True)
            nc.vector.tensor_copy(srow[0:1, h * 512:(h + 1) * 512], sp[:])
        # 1/sqrt(sumsq) = exp(-0.5*ln(sumsq)); eps in the reference
        # denominator is negligible (norm >> 1e-5 for random features)
        lnrow = sb_srow.tile([1, csz], F32, tag="lnrow")
        nc.scalar.activation(lnrow[:], srow[:], AF.Ln)
        rrow_r = sb_srow.tile([1, csz], F32R, tag="rrow_r")
        nc.scalar.activation(rrow_r[:], lnrow[:], AF.Exp, scale=-0.5)
        # broadcast 1/s across channel partitions via K=1 matmul
        for h in range(csz // 512):
            pc = pc2 * (csz // 512) + h
            bp = ps1.tile([CQ, 512], F32, tag="ps1")
            nc.tensor.matmul(bp[:], consts.ones_row_cq[:],
                             rrow_r[0:1, h * 512:(h + 1) * 512],
                             start=True, stop=True)
            nc.vector.tensor_tensor(out=feat_tile[0:CQ, pc * 512:(pc + 1) * 512],
                                    in0=f[:, pc * 512:(pc + 1) * 512],
                                    in1=bp[:], op=ALU.mult)
    # feat_tile rows 0:CQ now hold fp32 features; row CQ is the caller's
    # softmax-offset slot (fa) / ones slot (fb)


class _Consts:
    pass


def build():
    nc = bacc.Bacc("TRN2", target_bir_lowering=False, debug=False)
    fa = nc.dram_tensor("fa_roll", [N, C, HW], F32R, kind="ExternalInput").ap()
    fb = nc.dram_tensor("fb_raw", [N, C, HW], F32R, kind="ExternalInput").ap()
    fcT = nc.dram_tensor("fcT", [N, HW, C], F32R, kind="ExternalInput").ap()
    ones_hw = nc.dram_tensor("ones_hw", [1, HW], F32R, kind="ExternalInput").ap()
    WaT = nc.dram_tensor("WaT", [C, CQ], F32, kind="ExternalInput").ap()
    WbT = nc.dram_tensor("WbT", [C, CQ], F32, kind="ExternalInput").ap()
    corr_d = nc.dram_tensor("corr", [N, HW, QS], F32, kind="ExternalOutput").ap()
    warp_d = nc.dram_tensor("warp", [N, C, QS], F32, kind="ExternalOutput").ap()

    with tile.TileContext(nc) as tc:
        import contextlib
        ctx = contextlib.ExitStack()
        with ctx:
            sb_x = ctx.enter_context(tc.tile_pool(name="x", bufs=6))
            sb_y = ctx.enter_context(tc.tile_pool(name="y", bufs=3))
            sb_zf = ctx.enter_context(tc.tile_pool(name="zf", bufs=3))
            sb_scr = ctx.enter_context(tc.tile_pool(name="scr", bufs=3))
            sb_small = ctx.enter_context(tc.tile_pool(name="small", bufs=3))
            sb_srow = ctx.enter_context(tc.tile_pool(name="srow", bufs=2))
            sb_feat = ctx.enter_context(tc.tile_pool(name="feat", bufs=2))
            sb_fafeat = ctx.enter_context(tc.tile_pool(name="fafeat", bufs=2))
            sb_corr = ctx.enter_context(tc.tile_pool(name="corr", bufs=3))
            sb_fc = ctx.enter_context(tc.tile_pool(name="fc", bufs=4))
            sb_warp = ctx.enter_context(tc.tile_pool(name="warp", bufs=1))
            sb_const = ctx.enter_context(tc.tile_pool(name="const", bufs=1))
            ps1 = ctx.enter_context(tc.tile_pool(name="ps1", bufs=2, space="PSUM"))
            psC = ctx.enter_context(tc.tile_pool(name="psC", bufs=2, space="PSUM"))
            psB = ctx.enter_context(tc.tile_pool(name="psB", bufs=2, space="PSUM"))
            psW = ctx.enter_context(tc.tile_pool(name="psW", bufs=2, space="PSUM"))

            # ---- constants ----
            consts = _Consts()
            ident = sb_const.tile([128, 128], F32, tag="ident")
            masks.make_identity(nc, ident[:])
            ones_f32 = sb_const.tile([1, 128], F32, tag="ones_f32")
            nc.vector.memset(ones_f32[:], 1.0)
            ones_col_f32 = sb_const.tile([CQ, 1], F32, tag="ones_col_f32")
            nc.vector.memset(ones_col_f32[:], 1.0)
            neg100 = sb_const.tile([128, 1], F32, tag="neg100")
            nc.vector.memset(neg100[:], -ALPHA)

            ones_cq = sb_const.tile([CQ, 1], F32R, tag="ones_cq")
            nc.vector.tensor_copy(ones_cq[:], ones_col_f32[:])
            ones_row_cq = sb_const.tile([1, CQ], F32R, tag="ones_row_cq")
            nc.vector.tensor_copy(ones_row_cq[:], ones_f32[0:1, 0:CQ])

            consts.ones_cq = ones_cq
            consts.ones_row_cq = ones_row_cq

            # conv weights (transposed on host): [C, CQ] -> two [128, CQ] f32r
            WT_r = {"a": [], "b": []}
            for key, Wd in (("a", WaT), ("b", WbT)):
                for cc in range(2):
                    wt = sb_const.tile([128, CQ], F32, tag=f"w_{key}{cc}")
                    nc.sync.dma_start(wt[:], Wd[cc * 128:(cc + 1) * 128, :])
                    wtr = sb_const.tile([128, CQ], F32R, tag=f"wr_{key}{cc}")
                    nc.vector.tensor_copy(wtr[:], wt[:])
                    WT_r[key].append(wtr)

            
            pools_feat = (sb_x, sb_y, sb_zf, sb_scr, sb_small, sb_srow,
                          consts, ps1, psC)

            for n in range(N):
                # ---- features ----
                fa_feat = sb_fafeat.tile([CQ + 1, QS], F32R, tag="fafeat")
                _feat_pipeline(nc, tc, pools_feat, n, fa, WT_r["a"], fa_feat, QS)
                fb_feat = sb_feat.tile([CQ + 1, HW], F32R, tag="feat")
                _feat_pipeline(nc, tc, pools_feat, n, fb, WT_r["b"], fb_feat, HW)


                # ---- pass A: softmax stats in [q, k] layout ----
                Dstack = sb_small.tile([128, 4], F32, tag="Dstack")
                for j in range(QS // 128):
                    M8 = sb_small.tile([128, 8], F32, tag="M8")
                    S8 = sb_small.tile([128, 8], F32, tag="S8")
                    for kc in range(8):
                        sp = ps1.tile([128, 512], F32, tag="ps1")
                        nc.tensor.matmul(
                            sp[:], fa_feat[0:CQ, j * 128:(j + 1) * 128],
                            fb_feat[0:CQ, kc * 512:(kc + 1) * 512],
                            start=True, stop=True)
                        # fused chunk stats: out=-100*e (throwaway),
                        # accum(min) = -100*max(e) = the exp bias directly
                        biasc = sb_small.tile([128, 1], F32, tag="biasc")
                        scr0 = sb_scr.tile([128, 512], BF16, tag="scr")
                        nc.vector.tensor_scalar(out=scr0[:], in0=sp[:],
                                                scalar1=-ALPHA, scalar2=3.0e38,
                                                op0=ALU.mult, op1=ALU.min,
                                                accum_out=biasc[:])
                        scr = sb_scr.tile([128, 512], BF16, tag="scr")
                        nc.scalar.activation(scr[:], sp[:], AF.Exp,
                                             bias=biasc[:], scale=ALPHA,
                                             accum_out=S8[:, kc:kc + 1])
                        nc.vector.tensor_scalar_mul(M8[:, kc:kc + 1],
                                                    biasc[:], -1.0 / ALPHA)
                    M = sb_small.tile([128, 1], F32, tag="M")
                    nc.vector.reduce_max(M[:], M8[:], axis=mybir.AxisListType.X)
                    dlt = sb_small.tile([128, 8], F32, tag="dlt")
                    nc.vector.tensor_scalar(out=dlt[:], in0=M8[:], scalar1=M[:],
                                            scalar2=None, op0=ALU.subtract)
                    expd = sb_small.tile([128, 8], F32, tag="expd")
                    nc.scalar.activation(expd[:], dlt[:], AF.Exp, scale=ALPHA)
                    Sw = sb_small.tile([128, 8], F32, tag="Sw")
                    nc.vector.tensor_tensor(out=Sw[:], in0=S8[:], in1=expd[:],
                                            op=ALU.mult)
                    S = sb_small.tile([128, 1], F32, tag="S")
                    nc.vector.reduce_sum(S[:], Sw[:], axis=mybir.AxisListType.X)
                    lnS = sb_small.tile([128, 1], F32, tag="lnS")
                    nc.scalar.activation(lnS[:], S[:], AF.Ln)
                    # aug = -M - lnS/100 (psum = dot - D/100, D = 100M + lnS)
                    nc.vector.tensor_scalar(out=Dstack[:, j:j + 1], in0=lnS[:],
                                            scalar1=-1.0 / ALPHA, scalar2=None,
                                            op0=ALU.mult)
                    nc.vector.tensor_sub(Dstack[:, j:j + 1],
                                         Dstack[:, j:j + 1], M[:])
                # fb~ row CQ = 1.0, DMA'd from host-provided f32r ones
                nc.sync.dma_start(fb_feat[CQ:CQ + 1, :], ones_hw)
                # transpose Dstack [128, 4] -> [4, 128], stage to SBUF, then
                # assemble the [1, 512] offset row on partition 0 via DMA
                tp = ps1.tile([128, 512], F32, tag="ps1")
                nc.tensor.transpose(tp[0:4, 0:128], Dstack[:], ident[:])
                dstage = sb_small.tile([4, 128], F32, tag="dstage")
                nc.vector.tensor_copy(dstage[:], tp[0:4, 0:128])
                for j in range(QS // 128):
                    nc.sync.dma_start(fa_feat[CQ:CQ + 1, j * 128:(j + 1) * 128],
                                      dstage[j:j + 1, :].bitcast(F32R))

                # ---- pass B + warp ----
                wps = [psW.tile([128, 512], F32, tag="psW", name=f"wps{ct}")
                       for ct in range(2)]
                for g in range(NKT // KGRP):           # 8 groups of 4 k-tiles
                    corr_sb = sb_corr.tile([128, KGRP, 512], F32R, tag="corr")
                    for u in range(KGRP):
                        kt = g * KGRP + u
                        cp = psB.tile([128, 512], F32, tag="psB")
                        nc.tensor.matmul(
                            cp[:],
                            fb_feat[0:CQ + 1, kt * 128:(kt + 1) * 128],
                            fa_feat[0:CQ + 1, 0:QS],
                            start=True, stop=True)
                        nc.scalar.activation(
                            corr_sb[:, u, :], cp[:], AF.Exp, scale=ALPHA)
                    for u in range(KGRP):
                        kt = g * KGRP + u
                        fctr = sb_fc.tile([128, C], F32R, tag="fc")
                        nc.sync.dma_start(fctr[:],
                                          fcT[n, kt * 128:(kt + 1) * 128, :])
                        for ct in range(2):
                            nc.tensor.matmul(
                                wps[ct][:], fctr[:, ct * 128:(ct + 1) * 128],
                                corr_sb[:, u, :],
                                start=(kt == 0), stop=(kt == NKT - 1))
                    nc.sync.dma_start(
                        corr_d[n, g * KGRP * 128:(g + 1) * KGRP * 128, :]
                        .rearrange("(j p) q -> p j q", p=128),
                        corr_sb[:].bitcast(F32))
                warp_sb = sb_warp.tile([128, 2, 512], F32, tag="warp")
                for ct in range(2):
                    nc.vector.tensor_copy(warp_sb[:, ct, :], wps[ct][:])
                nc.sync.dma_start(
                    warp_d[n].rearrange("(ct p) q -> p ct q", p=128),
                    warp_sb[:])

    nc.compile()
    return nc


_NC_CACHE = None


def _get_nc():
    global _NC_CACHE
    if _NC_CACHE is None:
        _NC_CACHE = build()
    return _NC_CACHE


def make_in_maps(fa_raw, fb_raw, fc_raw, Wa, ba, Wb, bb):
    """Host-side marshalling. ba/bb provably cancel in instance norm."""
    fa2 = np.ascontiguousarray(fa_raw.reshape(N, C, HW), dtype=np.float32)
    fb2 = np.ascontiguousarray(fb_raw.reshape(N, C, HW), dtype=np.float32)
    fcT = np.ascontiguousarray(
        fc_raw.reshape(N, C, HW).transpose(0, 2, 1), dtype=np.float32)
    WaT = np.ascontiguousarray(Wa.T, dtype=np.float32)
    WbT = np.ascontiguousarray(Wb.T, dtype=np.float32)
    ones_hw = np.ones((1, HW), dtype=np.float32)
    in_maps = []
    for core in range(NCORES):
        fa_roll = np.ascontiguousarray(np.roll(fa2, -core * QS, axis=2))
        in_maps.append(dict(fa_roll=fa_roll, fb_raw=fb2, fcT=fcT,
                            WaT=WaT, WbT=WbT, ones_hw=ones_hw))
    return in_maps


LAST_RESULTS = None


def kernel(fa_raw, fb_raw, fc_raw, Wa, ba, Wb, bb):
    global LAST_RESULTS
    nc = _get_nc()
    in_maps = make_in_maps(fa_raw, fb_raw, fc_raw, Wa, ba, Wb, bb)
    res = run_bass_kernel_spmd(nc, in_maps, core_ids=list(range(NCORES)))
    LAST_RESULTS = res
    corr = np.concatenate([res.results[c]["corr"] for c in range(NCORES)],
                          axis=2)
    warp = np.concatenate([res.results[c]["warp"] for c in range(NCORES)],
                          axis=2)
    return warp.reshape(N, C, 64, 64), corr
